# revision 1
# baseline (speedup 1.0000x reference)
"""3-layer GraphSAGE (mean aggr) on Trainium2, 8-core SPMD, fp16 compute.

Strategy (graph/data parallel per the sharding hint):
  - Nodes padded 10000 -> 10240 and assigned to 80 blocks of 128 by a
    degree-balancing permutation (host-side index work), so every block
    has ~2000 in-edges and needs exactly 16 edge chunks. Core r owns
    blocks [10r, 10r+10); one SPMD program runs on all 8 cores.
  - Per layer, source rows are fetched with GPSIMD dma_gather (fp16,
    256-512B rows; multi-packet mode -- single_packet crashes >8 chunks).
    Segment-mean runs on the PE as one-hot matmuls: gathered chunk rows
    as lhsT, a one-hot built on-device (iota==dst_local, 1/deg folded in,
    cached in SBUF and reused across layers) as rhs, accumulating mean^T
    in f32 PSUM. mean^T then feeds the layer matmul directly as lhsT.
  - Layer 2 emits h2 TRANSPOSED (w2l k-slices as lhsT), so h2 never
    touches DRAM; t3 = h2@[w3l|w3r] (layer-3 "transform first") runs in
    the same block loop and only t3 [10240,128pad] fp16 is AllGathered,
    never h2 [10240,1024]. The root half w2r.h1^T is precomputed into
    SBUF while the h1 AllGather is in flight (it only needs local data)
    and added back on DVE, shortening the post-AllGather critical path.
  - h1^T, r3, and the one-hot cache stay SBUF-resident across phases.
    Cross-core traffic is two AllGathers: h1 (5.2MB fp16), t3 (2.6MB).
  - log_softmax per node on DVE/ACT in f32; per-core output slices are
    concatenated and un-permuted on host. PSUM accumulation is f32;
    end-to-end vs the f32 reference: max abs err ~2e-3 (rel ~2.6e-4).
"""

import numpy as np
BF = np.float16

N_NODES = 10000
NPAD = 10240
NCORES = 8
P = 128
NB = 10                      # dst blocks per core
PER_CORE = NB * P            # 1280
D_IN, D_H1, D_H2, D_OUT = 128, 256, 1024, 64

_CACHE = {}
LAST_RESULTS = None          # test harness reads exec_time_ns from here


def _build(MC):
    import os
    import concourse.bacc as bacc
    import concourse.mybir as mybir
    import concourse.tile as tile

    abl = set(os.environ.get("KABL", "").split(","))

    f32 = mybir.dt.float32
    bf16 = mybir.dt.float16
    i16 = mybir.dt.int16
    nc = bacc.Bacc("TRN2", target_bir_lowering=False, debug=False,
                   num_devices=NCORES)

    xbf = nc.dram_tensor("xbf", [NPAD, D_IN], bf16, kind="ExternalInput")
    xownT = nc.dram_tensor("xownT", [P, PER_CORE], bf16, kind="ExternalInput")
    w1l = nc.dram_tensor("w1l", [D_IN, D_H1], bf16, kind="ExternalInput")
    w1r = nc.dram_tensor("w1r", [D_IN, D_H1], bf16, kind="ExternalInput")
    b1 = nc.dram_tensor("b1", [1, D_H1], bf16, kind="ExternalInput")
    b1t = nc.dram_tensor("b1t", [P, 2], f32, kind="ExternalInput")
    w2l = nc.dram_tensor("w2l", [D_H1, D_H2], bf16, kind="ExternalInput")
    w2r = nc.dram_tensor("w2r", [D_H1, D_H2], bf16, kind="ExternalInput")
    b2t = nc.dram_tensor("b2t", [P, 8], f32, kind="ExternalInput")
    w3lr = nc.dram_tensor("w3lr", [D_H2, P], bf16, kind="ExternalInput")
    b3pad = nc.dram_tensor("b3pad", [1, P], bf16, kind="ExternalInput")
    gidx = nc.dram_tensor("gidx", [P, NB * MC * 8], i16, kind="ExternalInput")
    dstloc = nc.dram_tensor("dstloc", [P, NB * MC], f32, kind="ExternalInput")
    invdeg = nc.dram_tensor("invdeg", [P, NB * MC], f32, kind="ExternalInput")
    iota_in = nc.dram_tensor("iota_in", [P, P], f32, kind="ExternalInput")
    outp = nc.dram_tensor("out", [PER_CORE, D_OUT], f32, kind="ExternalOutput")

    EXP = mybir.ActivationFunctionType.Exp
    LN = mybir.ActivationFunctionType.Ln
    RELU = mybir.ActivationFunctionType.Relu
    EQ = mybir.AluOpType.is_equal
    MUL = mybir.AluOpType.mult
    SUB = mybir.AluOpType.subtract
    ADD = mybir.AluOpType.add
    MAX = mybir.AluOpType.max
    AXX = mybir.AxisListType.X

    with tile.TileContext(nc) as tc:
        with (
            tc.tile_pool(name="const", bufs=1) as cp,
            tc.tile_pool(name="gath", bufs=3) as gp,
            tc.tile_pool(name="ht", bufs=3) as htp,
            tc.tile_pool(name="meant", bufs=3) as mtp,
            tc.tile_pool(name="hout", bufs=2) as hop,
            tc.tile_pool(name="small", bufs=6) as smp,
            tc.tile_pool(name="psA", bufs=2, space="PSUM") as psA,
            tc.tile_pool(name="psO", bufs=2, space="PSUM") as psO,
            tc.tile_pool(name="psT", bufs=4, space="PSUM") as psT,
            tc.tile_pool(name="dram", bufs=1, space="DRAM") as dram,
        ):
            # ---- constants ----
            iota_t = cp.tile([P, P], f32, tag="iota")
            nc.sync.dma_start(iota_t[:], iota_in[:])
            ones_t = cp.tile([1, P], bf16, tag="ones")
            nc.vector.memset(ones_t[:], 1.0)
            gidx_sb = cp.tile([P, NB * MC * 8], i16, tag="gidx")
            nc.sync.dma_start(gidx_sb[:], gidx[:])
            dstloc_sb = cp.tile([P, NB * MC], f32, tag="dstloc")
            nc.sync.dma_start(dstloc_sb[:], dstloc[:])
            invdeg_sb = cp.tile([P, NB * MC], f32, tag="invdeg")
            nc.sync.dma_start(invdeg_sb[:], invdeg[:])

            # weights arrive fp16 from the host; plain HWDGE loads
            w1l_sb = cp.tile([P, D_H1], bf16, tag="w1l")
            nc.sync.dma_start(w1l_sb[:], w1l[:])
            w1r_sb = cp.tile([P, D_H1], bf16, tag="w1r")
            nc.sync.dma_start(w1r_sb[:], w1r[:])
            b1_sb = cp.tile([1, D_H1], bf16, tag="b1")
            nc.sync.dma_start(b1_sb[:], b1[:])
            b1t_sb = cp.tile([P, 2], f32, tag="b1t")
            nc.sync.dma_start(b1t_sb[:], b1t[:])

            # resident cross-phase SBUF state
            xT_res = cp.tile([P, PER_CORE], bf16, tag="xT")
            nc.sync.dma_start(xT_res[:], xownT[:])
            h1T_res = cp.tile([P, 2, PER_CORE], bf16, tag="h1T")
            r3_res = cp.tile([P, NB, D_OUT], f32, tag="r3")
            ohc = cp.tile([P, NB * MC, P], bf16, tag="ohc")

            # ---- DRAM intermediates ----
            h1_own = dram.tile([PER_CORE, D_H1], bf16, tag="h1o")
            h1_full = dram.tile([NPAD, D_H1], bf16, tag="h1f")
            t3_own = dram.tile([PER_CORE, P], bf16, tag="t3o")
            t3_full = dram.tile([NPAD, P], bf16, tag="t3f")

            # single_packet=True caps one gather at 64 descs/engine (8
            # chunks) and crashes beyond; multi-packet handles 18+ chunks.
            # 8-chunk sub-gathers measured fastest (finer DMA/compute
            # interleave at phase starts vs per-gather fixed overhead).
            def gather_block(dst_tile, src_ap, b, elem, GSZ=8):
                if "nogather" in abl:
                    return
                g0 = 0
                while g0 < MC:
                    gsz = min(GSZ, MC - g0)
                    c0 = (b * MC + g0) * 8
                    nc.gpsimd.dma_gather(
                        dst_tile[:, g0:g0 + gsz, :], src_ap,
                        gidx_sb[:, c0:c0 + gsz * 8],
                        gsz * P, gsz * P, elem, single_packet=False)
                    g0 += gsz

            # ================= Layer 1 =================
            for b in range(NB if "l1" not in abl else 0):
                gath = gp.tile([P, MC, D_IN], bf16, tag="gath")
                gather_block(gath, xbf[:], b, D_IN)
                # build + cache this block's one-hots (bf16, reused in L2)
                for c in range(MC if "nooh" not in abl else 0):
                    col = b * MC + c
                    nc.vector.tensor_scalar(
                        ohc[:, col, :], iota_t[:],
                        dstloc_sb[:, col:col + 1], invdeg_sb[:, col:col + 1],
                        EQ, MUL)
                agg = psA.tile([P, 2 * P], f32, tag="agg")
                for c in range(MC if "noagg" not in abl else 0):
                    nc.tensor.matmul(agg[:, 0:P], gath[:, c, :],
                                     ohc[:, b * MC + c, :],
                                     start=(c == 0), stop=(c == MC - 1))
                meanT = mtp.tile([P, 2, P], bf16, tag="meanT")
                nc.vector.tensor_copy(meanT[:, 0, :], agg[:, 0:P])

                xT = xT_res[:, b * P:(b + 1) * P]

                # node-major h1 (for AllGather/gather)
                op = psO.tile([P, 2 * D_H1], f32, tag="outp")
                nc.tensor.matmul(op[:, 0:D_H1], meanT[:, 0, :], w1l_sb[:],
                                 start=True, stop=False)
                nc.tensor.matmul(op[:, 0:D_H1], xT, w1r_sb[:],
                                 start=False, stop=False)
                nc.tensor.matmul(op[:, 0:D_H1], ones_t[:], b1_sb[:],
                                 start=False, stop=True)
                h1blk = hop.tile([P, D_H1], bf16, tag="hout")
                nc.scalar.activation(h1blk[:], op[:, 0:D_H1], RELU)
                nc.sync.dma_start(h1_own[b * P:(b + 1) * P, :], h1blk[:])
                # transposed h1 (for the L2 root term), SBUF-resident
                for s in range(2):
                    tph = psT.tile([P, P], f32, tag="tp")
                    nc.tensor.matmul(tph[:], w1l_sb[:, s * P:(s + 1) * P],
                                     meanT[:, 0, :], start=True, stop=False)
                    nc.tensor.matmul(tph[:], w1r_sb[:, s * P:(s + 1) * P],
                                     xT, start=False, stop=True)
                    nc.scalar.activation(h1T_res[:, s, b * P:(b + 1) * P],
                                         tph[:], RELU,
                                         bias=b1t_sb[:, s:s + 1])

            # layer-2/3 weights are first needed after L1: emit their loads
            # here so they don't delay the L1 gathers in the SWDGE queue
            w2l_sb = cp.tile([P, 2, D_H2], bf16, tag="w2l")
            nc.sync.dma_start(w2l_sb[:], w2l.rearrange("(s p) n -> p s n", p=P))
            w2r_sb = cp.tile([P, 2, D_H2], bf16, tag="w2r")
            nc.sync.dma_start(w2r_sb[:], w2r.rearrange("(s p) n -> p s n", p=P))
            b2t_sb = cp.tile([P, 8], f32, tag="b2t")
            nc.sync.dma_start(b2t_sb[:], b2t[:])
            w3lr_sb = cp.tile([P, 8, P], bf16, tag="w3lr")
            nc.sync.dma_start(w3lr_sb[:], w3lr.rearrange("(s p) n -> p s n", p=P))
            b3_sb = cp.tile([1, P], bf16, tag="b3")
            nc.sync.dma_start(b3_sb[:], b3pad[:])

            # root half of h2^T depends only on local h1T_res: compute it
            # while the h1 AllGather is in flight, staged to SBUF.
            h2root = cp.tile([P, NB, 8, P], f32, tag="h2r")
            for b in range(NB if "l2" not in abl else 0):
                for s in range(8):
                    hp = psT.tile([P, P], f32, tag="tp")
                    for k in range(2):
                        nc.tensor.matmul(
                            hp[:], w2r_sb[:, k, s * P:(s + 1) * P],
                            h1T_res[:, k, b * P:(b + 1) * P],
                            start=(k == 0), stop=(k == 1))
                    nc.vector.tensor_copy(h2root[:, b, s, :], hp[:])

            if "noag" not in abl:
                nc.gpsimd.collective_compute(
                    "AllGather", mybir.AluOpType.bypass,
                    replica_groups=[list(range(NCORES))],
                    ins=[h1_own.opt()], outs=[h1_full.opt()])

            # ====== Layer 2 fused with L3 transform (h2 never in DRAM) =====
            for b in range(NB if "l2" not in abl else 0):
                gath = gp.tile([P, MC, D_H1], bf16, tag="gath")
                gather_block(gath, h1_full[:], b, D_H1, GSZ=6)
                agg = psA.tile([P, 2 * P], f32, tag="agg")
                for s in range(2 if "noagg" not in abl else 0):
                    for c in range(MC):
                        nc.tensor.matmul(agg[:, s * P:(s + 1) * P],
                                         gath[:, c, s * P:(s + 1) * P],
                                         ohc[:, b * MC + c, :],
                                         start=(c == 0), stop=(c == MC - 1))
                meanT = mtp.tile([P, 2, P], bf16, tag="meanT")
                nc.vector.tensor_copy(meanT[:, 0, :], agg[:, 0:P])
                nc.vector.tensor_copy(meanT[:, 1, :], agg[:, P:2 * P])

                # h2^T slices: mean half on PE, + staged root half on DVE
                hT = htp.tile([P, 8, P], bf16, tag="hT")
                for s in range(8 if "noout" not in abl else 0):
                    hp = psT.tile([P, P], f32, tag="tp")
                    for k in range(2):
                        nc.tensor.matmul(
                            hp[:], w2l_sb[:, k, s * P:(s + 1) * P],
                            meanT[:, k, :], start=(k == 0), stop=(k == 1))
                    ysum = smp.tile([P, P], f32, tag="ysum")
                    nc.vector.tensor_tensor(ysum[:], hp[:],
                                            h2root[:, b, s, :], ADD)
                    nc.scalar.activation(hT[:, s, :], ysum[:], RELU,
                                         bias=b2t_sb[:, s:s + 1])

                # [t3 | r3] = h2 @ [w3l | w3r] + [0 | b3] (lhsT = h2^T)
                tr = psO.tile([P, 2 * D_H1], f32, tag="outp")
                for s in range(8):
                    nc.tensor.matmul(tr[:, 0:P], hT[:, s, :],
                                     w3lr_sb[:, s, :],
                                     start=(s == 0), stop=False)
                nc.tensor.matmul(tr[:, 0:P], ones_t[:], b3_sb[:],
                                 start=False, stop=True)
                t3blk = smp.tile([P, P], bf16, tag="t3blk")
                nc.vector.tensor_copy(t3blk[:, 0:D_OUT], tr[:, 0:D_OUT])
                nc.vector.memset(t3blk[:, D_OUT:P], 0.0)
                nc.vector.tensor_copy(r3_res[:, b, :], tr[:, D_OUT:P])
                nc.sync.dma_start(t3_own[b * P:(b + 1) * P, :], t3blk[:])

            if "noag" not in abl:
                nc.gpsimd.collective_compute(
                    "AllGather", mybir.AluOpType.bypass,
                    replica_groups=[list(range(NCORES))],
                    ins=[t3_own.opt()], outs=[t3_full.opt()])

            # ================= Layer 3 aggregate + log_softmax ==============
            for b in range(NB if "l3b" not in abl else 0):
                gath = gp.tile([P, MC, P], bf16, tag="gath")
                gather_block(gath, t3_full[:], b, P)
                agg = psA.tile([P, 2 * P], f32, tag="agg")
                for c in range(MC if "noagg" not in abl else 0):
                    nc.tensor.matmul(agg[:, 0:D_OUT], ohc[:, b * MC + c, :],
                                     gath[:, c, 0:D_OUT],
                                     start=(c == 0), stop=(c == MC - 1))
                y = smp.tile([P, D_OUT], f32, tag="y")
                nc.vector.tensor_tensor(y[:], agg[:, 0:D_OUT],
                                        r3_res[:, b, :], ADD)
                negm = smp.tile([P, 1], f32, tag="negm")
                nc.vector.tensor_reduce(negm[:], y[:], AXX, MAX, negate=True)
                e = smp.tile([P, D_OUT], f32, tag="e")
                ssum = smp.tile([P, 1], f32, tag="ssum")
                nc.scalar.activation(e[:], y[:], EXP, bias=negm[:, 0:1],
                                     scale=1.0, accum_out=ssum[:])
                ls = smp.tile([P, 1], f32, tag="ls")
                nc.scalar.activation(ls[:], ssum[:], LN)
                ob = smp.tile([P, D_OUT], f32, tag="ob")
                nc.vector.tensor_scalar(ob[:], y[:], negm[:, 0:1],
                                        ls[:, 0:1], ADD, SUB)
                nc.sync.dma_start(outp[b * P:(b + 1) * P, :], ob[:])

    nc.compile()
    return nc


def _wrap16(a):
    """idx i -> partition i%16, col i//16; replicated to 128 partitions."""
    w = a.reshape(-1, 16).T
    return np.ascontiguousarray(np.tile(w, (8, 1)))


def _balanced_perm(deg):
    """Assign nodes to 80 blocks of 128 so block in-degree sums are even.

    Greedy: highest-degree node goes to the open block with the lowest
    degree sum. Returns newpos[old_node] -> permuted node id. This makes
    every block need exactly ceil(E/(NBLK*P)) = 16 edge chunks instead of
    the unbalanced max (17+), trimming gather + aggregation work ~6%.
    """
    import heapq
    nblk = NPAD // P
    order = np.argsort(-deg, kind="stable")
    heap = [(0, 0, g) for g in range(nblk)]
    heapq.heapify(heap)
    newpos = np.empty(NPAD, np.int64)
    fill = np.zeros(nblk, np.int64)
    for n in order:
        s, _, g = heapq.heappop(heap)
        newpos[n] = g * P + fill[g]
        fill[g] += 1
        if fill[g] < P:
            heapq.heappush(heap, (s + int(deg[n]), int(fill[g]), g))
    return newpos


def _prep(x, edge_index):
    src = np.asarray(edge_index[0], dtype=np.int64)
    dst = np.asarray(edge_index[1], dtype=np.int64)
    deg = np.bincount(dst, minlength=NPAD).astype(np.float64)
    invdeg_n = (1.0 / np.maximum(deg, 1.0)).astype(np.float32)

    newpos = _balanced_perm(deg)
    oldnode = np.empty(NPAD, np.int64)
    oldnode[newpos] = np.arange(NPAD)
    psrc = newpos[src]
    pdst = newpos[dst]

    order = np.argsort(pdst, kind="stable")
    dsts = pdst[order]
    srcs = psrc[order]
    inv_e = invdeg_n[dst[order]]
    starts = np.searchsorted(dsts, np.arange(0, NPAD + P, P))
    cnt = starts[1:] - starts[:-1]
    MC = max(1, int(np.ceil(cnt.max() / P)))

    xp = np.zeros((NPAD, D_IN), dtype=np.float32)
    xp[:N_NODES] = x
    xp = xp[oldnode]           # permuted node order

    per_core = []
    for r in range(NCORES):
        gparts, dparts, iparts = [], [], []
        for j in range(NB):
            g = r * NB + j
            lo, hi = starts[g], starts[g + 1]
            n = hi - lo
            # ascending source addresses -> better HBM locality in the
            # gather's descriptor stream (aggregation is order-invariant)
            o2 = lo + np.argsort(srcs[lo:hi], kind="stable")
            sg = np.zeros(MC * P, dtype=np.int16)
            dg = np.full(MC * P, -1.0, dtype=np.float32)
            ig = np.zeros(MC * P, dtype=np.float32)
            sg[:n] = srcs[o2].astype(np.int16)
            dg[:n] = (dsts[o2] - g * P).astype(np.float32)
            ig[:n] = inv_e[o2]
            gparts.append(_wrap16(sg))
            dparts.append(np.ascontiguousarray(dg.reshape(MC, P).T))
            iparts.append(np.ascontiguousarray(ig.reshape(MC, P).T))
        per_core.append((
            np.concatenate(gparts, axis=1),
            np.concatenate(dparts, axis=1),
            np.concatenate(iparts, axis=1),
        ))
    return xp, per_core, MC, newpos


def _make_in_maps(x, edge_index, w1l, w1r, b1, w2l, w2r, b2, w3l, w3r, b3):
    x = np.ascontiguousarray(np.asarray(x, dtype=np.float32))
    xp, per_core, MC, newpos = _prep(x, np.asarray(edge_index))

    iota = np.tile(np.arange(P, dtype=np.float32), (P, 1))
    b1v = np.asarray(b1, np.float32).reshape(-1)
    b2v = np.asarray(b2, np.float32).reshape(-1)
    xbf = xp.astype(BF)
    common = {
        "xbf": xbf,
        "w1l": np.asarray(w1l, np.float32).astype(BF),
        "w1r": np.asarray(w1r, np.float32).astype(BF),
        "b1": b1v.reshape(1, D_H1).astype(BF),
        "b1t": np.ascontiguousarray(b1v.reshape(2, P).T),
        "w2l": np.asarray(w2l, np.float32).astype(BF),
        "w2r": np.asarray(w2r, np.float32).astype(BF),
        "b2t": np.ascontiguousarray(b2v.reshape(8, P).T),
        "w3lr": np.ascontiguousarray(np.concatenate(
            [np.asarray(w3l, np.float32), np.asarray(w3r, np.float32)],
            axis=1)).astype(BF),
        "b3pad": np.concatenate(
            [np.zeros(D_OUT, np.float32),
             np.asarray(b3, np.float32).reshape(-1)]).reshape(1, P).astype(BF),
        "iota_in": iota,
    }
    in_maps = []
    for r in range(NCORES):
        g, d, iv = per_core[r]
        m = dict(common)
        m["xownT"] = np.ascontiguousarray(
            xbf[r * PER_CORE:(r + 1) * PER_CORE].T)
        m["gidx"] = g
        m["dstloc"] = d
        m["invdeg"] = iv
        in_maps.append(m)
    return in_maps, MC, newpos


def kernel(x, edge_index, w1l, w1r, b1, w2l, w2r, b2, w3l, w3r, b3):
    global LAST_RESULTS
    import os
    from concourse.bass_utils import run_bass_kernel_spmd

    if os.environ.get("BASS_TRACE"):
        try:
            import antenv.axon_hooks  # noqa: F401
        except ImportError:
            os.environ.pop("BASS_TRACE", None)  # no NTFF hook here

    in_maps, MC, newpos = _make_in_maps(x, edge_index, w1l, w1r, b1, w2l,
                                        w2r, b2, w3l, w3r, b3)
    if MC not in _CACHE:
        _CACHE[MC] = _build(MC)
    nc = _CACHE[MC]

    res = run_bass_kernel_spmd(nc, in_maps, core_ids=list(range(NCORES)))
    LAST_RESULTS = res
    out = np.concatenate([res.results[r]["out"] for r in range(NCORES)], axis=0)
    return np.ascontiguousarray(out[newpos[:N_NODES]])



# revision 4
# speedup vs baseline: 1.1395x; 1.1395x over previous
"""3-layer GraphSAGE (mean aggr) on Trainium2, 8-core SPMD, fp16 compute.

Strategy (graph/data parallel, ReduceScatter formulation):
  - Nodes padded 10000 -> 10240, degree-balanced permutation into 80
    blocks of 128 (block in-degrees ~equal so L1 needs exactly 16 edge
    chunks per block). Core r owns blocks [10r, 10r+10).
  - L1 (x replicated, no exchange): dst-side gather of x rows per owned
    block + one-hot matmul segment-mean on the PE; produces node-major
    h1_own (DRAM, for L2 gathers) and feat-major h1T (SBUF, for the L2
    root term).
  - L2/L3 use SRC-side partial aggregation + ReduceScatter instead of
    AllGather: core r takes the ~20k edges whose SOURCE it owns, grouped
    by dst block (80 groups x MC2 chunks of 128, padded), gathers its
    own h1 rows per edge, and accumulates per-dst-block partial sums
    (one-hot matmuls, invdeg folded in) into a [10240,256] fp16 partial
    tensor. One ReduceScatter(add) then gives each core the exact mean
    rows for its own 1280 nodes.  Collective cost drops ~5x vs AllGather
    (charged on the 0.66MB scattered output, not the 5.2MB gathered one).
  - L2 transform consumes the scattered mean directly (PE transpose via
    identity matmul to feat-major), folds the root half as extra
    accumulating matmuls from SBUF-resident h1T, and emits h2 transposed
    so t3|r3 = h2 @ [w3l|w3r] runs in the same block loop (h2 never in
    DRAM).  L3 repeats the src-side partial + ReduceScatter on the
    64-wide t3 (padded to 128 cols for the 256B-min gather row size).
  - log_softmax: all EXP passes first (accumulating per-block sums),
    then ONE Ln over [128,10] — avoids per-block activation-table
    thrash.  PSUM accumulation is f32 end-to-end.
"""

import numpy as np
BF = np.float16

N_NODES = 10000
NPAD = 10240
NCORES = 8
P = 128
NB = 10                      # dst blocks per core
NBLK = NPAD // P             # 80
PER_CORE = NB * P            # 1280
D_IN, D_H1, D_H2, D_OUT = 128, 256, 1024, 64

_CACHE = {}
LAST_RESULTS = None          # test harness reads exec_time_ns from here


def _build(key):
    import os
    import concourse.bacc as bacc
    import concourse.mybir as mybir
    import concourse.tile as tile

    MC, MC2 = key
    abl = set(os.environ.get("KABL", "").split(","))
    GG = int(os.environ.get("KGG", "8"))      # dst blocks per src-side gather

    f32 = mybir.dt.float32
    bf16 = mybir.dt.float16
    i16 = mybir.dt.int16
    nc = bacc.Bacc("TRN2", target_bir_lowering=False, debug=False,
                   num_devices=NCORES)

    xbf = nc.dram_tensor("xbf", [NPAD, D_IN], bf16, kind="ExternalInput")
    xownT = nc.dram_tensor("xownT", [P, PER_CORE], bf16, kind="ExternalInput")
    w1l = nc.dram_tensor("w1l", [D_IN, D_H1], bf16, kind="ExternalInput")
    w1r = nc.dram_tensor("w1r", [D_IN, D_H1], bf16, kind="ExternalInput")
    b1 = nc.dram_tensor("b1", [1, D_H1], bf16, kind="ExternalInput")
    b1t = nc.dram_tensor("b1t", [P, 2], f32, kind="ExternalInput")
    w2l = nc.dram_tensor("w2l", [D_H1, D_H2], bf16, kind="ExternalInput")
    w2r = nc.dram_tensor("w2r", [D_H1, D_H2], bf16, kind="ExternalInput")
    b2t = nc.dram_tensor("b2t", [P, 8], f32, kind="ExternalInput")
    w3lr = nc.dram_tensor("w3lr", [D_H2, P], bf16, kind="ExternalInput")
    b3pad = nc.dram_tensor("b3pad", [1, P], bf16, kind="ExternalInput")
    gidx = nc.dram_tensor("gidx", [P, NB * MC * 8], i16, kind="ExternalInput")
    dstloc = nc.dram_tensor("dstloc", [P, NB * MC], f32, kind="ExternalInput")
    invdeg = nc.dram_tensor("invdeg", [P, NB * MC], f32, kind="ExternalInput")
    gidx2 = nc.dram_tensor("gidx2", [P, NBLK * MC2 * 8], i16,
                           kind="ExternalInput")
    dstloc2 = nc.dram_tensor("dstloc2", [P, NBLK * MC2], f32,
                             kind="ExternalInput")
    invdeg2 = nc.dram_tensor("invdeg2", [P, NBLK * MC2], f32,
                             kind="ExternalInput")
    iota_in = nc.dram_tensor("iota_in", [P, P], f32, kind="ExternalInput")
    ident_in = nc.dram_tensor("ident_in", [P, P], bf16, kind="ExternalInput")
    outp = nc.dram_tensor("out", [PER_CORE, D_OUT], f32, kind="ExternalOutput")

    EXP = mybir.ActivationFunctionType.Exp
    LN = mybir.ActivationFunctionType.Ln
    RELU = mybir.ActivationFunctionType.Relu
    COPY = mybir.ActivationFunctionType.Copy
    EQ = mybir.AluOpType.is_equal
    MUL = mybir.AluOpType.mult
    SUB = mybir.AluOpType.subtract
    ADD = mybir.AluOpType.add
    MAX = mybir.AluOpType.max
    AXX = mybir.AxisListType.X

    with tile.TileContext(nc) as tc:
        with (
            tc.tile_pool(name="const", bufs=1) as cp,
            tc.tile_pool(name="gath", bufs=3) as gp,
            tc.tile_pool(name="gath2", bufs=2) as gp2,
            tc.tile_pool(name="oh1", bufs=4) as ohp,
            tc.tile_pool(name="ht", bufs=2) as htp,
            tc.tile_pool(name="meant", bufs=3) as mtp,
            tc.tile_pool(name="hout", bufs=2) as hop,
            tc.tile_pool(name="small", bufs=6) as smp,
            tc.tile_pool(name="psA", bufs=2, space="PSUM") as psA,
            tc.tile_pool(name="psO", bufs=2, space="PSUM") as psO,
            tc.tile_pool(name="psT", bufs=3, space="PSUM") as psT,
            tc.tile_pool(name="dram", bufs=1, space="DRAM") as dram,
        ):
            # ---- constants ----
            iota_t = cp.tile([P, P], f32, tag="iota")
            nc.sync.dma_start(iota_t[:], iota_in[:])
            ident_t = cp.tile([P, P], bf16, tag="ident")
            nc.sync.dma_start(ident_t[:], ident_in[:])
            ones_t = cp.tile([1, P], bf16, tag="ones")
            nc.vector.memset(ones_t[:], 1.0)
            gidx_sb = cp.tile([P, NB * MC * 8], i16, tag="gidx")
            nc.sync.dma_start(gidx_sb[:], gidx[:])
            dstloc_sb = cp.tile([P, NB * MC], f32, tag="dstloc")
            nc.sync.dma_start(dstloc_sb[:], dstloc[:])
            invdeg_sb = cp.tile([P, NB * MC], f32, tag="invdeg")
            nc.sync.dma_start(invdeg_sb[:], invdeg[:])
            gidx2_sb = cp.tile([P, NBLK * MC2 * 8], i16, tag="gidx2")
            nc.sync.dma_start(gidx2_sb[:], gidx2[:])
            dstloc2_sb = cp.tile([P, NBLK * MC2], f32, tag="dstloc2")
            nc.sync.dma_start(dstloc2_sb[:], dstloc2[:])
            invdeg2_sb = cp.tile([P, NBLK * MC2], f32, tag="invdeg2")
            nc.sync.dma_start(invdeg2_sb[:], invdeg2[:])

            w1l_sb = cp.tile([P, D_H1], bf16, tag="w1l")
            nc.sync.dma_start(w1l_sb[:], w1l[:])
            w1r_sb = cp.tile([P, D_H1], bf16, tag="w1r")
            nc.sync.dma_start(w1r_sb[:], w1r[:])
            b1_sb = cp.tile([1, D_H1], bf16, tag="b1")
            nc.sync.dma_start(b1_sb[:], b1[:])
            b1t_sb = cp.tile([P, 2], f32, tag="b1t")
            nc.sync.dma_start(b1t_sb[:], b1t[:])

            # resident cross-phase SBUF state
            xT_res = cp.tile([P, PER_CORE], bf16, tag="xT")
            nc.sync.dma_start(xT_res[:], xownT[:])
            h1T_res = cp.tile([P, 2, PER_CORE], bf16, tag="h1T")
            r3_res = cp.tile([P, NB, D_OUT], f32, tag="r3")
            y_res = cp.tile([P, NB, D_OUT], f32, tag="y")
            negm_res = cp.tile([P, NB], f32, tag="negm")
            ssum_res = cp.tile([P, NB], f32, tag="ssum")
            ls_res = cp.tile([P, NB], f32, tag="ls")
            ohc2 = cp.tile([P, NBLK * MC2, P], bf16, tag="ohc2")

            # ---- DRAM intermediates ----
            h1_own = dram.tile([PER_CORE, D_H1], bf16, tag="h1o")
            t3_own = dram.tile([PER_CORE, P], bf16, tag="t3o")
            part2 = dram.tile([NPAD, D_H1], bf16, tag="p2")
            rs1out = dram.tile([PER_CORE, D_H1], bf16, tag="rs1")
            part3 = dram.tile([NPAD, D_OUT], bf16, tag="p3")
            rs2out = dram.tile([PER_CORE, D_OUT], bf16, tag="rs2")

            def gather_block(dst_tile, src_ap, idx_sb, c0, nchunk, elem,
                             GSZ=8):
                """Gather nchunk*128 rows starting at chunk c0 of idx_sb."""
                if "nogather" in abl:
                    return
                g0 = 0
                while g0 < nchunk:
                    gsz = min(GSZ, nchunk - g0)
                    cc = (c0 + g0) * 8
                    nc.gpsimd.dma_gather(
                        dst_tile[:, g0:g0 + gsz, :], src_ap,
                        idx_sb[:, cc:cc + gsz * 8],
                        gsz * P, gsz * P, elem, single_packet=False)
                    g0 += gsz

            # ================= Layer 1 (dst-side, local) =================
            for b in range(NB if "l1" not in abl else 0):
                gath = gp.tile([P, MC, D_IN], bf16, tag="gath")
                gather_block(gath, xbf[:], gidx_sb, b * MC, MC, D_IN)
                oh1 = ohp.tile([P, MC, P], bf16, tag="oh1")
                for c in range(MC):
                    col = b * MC + c
                    nc.vector.tensor_scalar(
                        oh1[:, c, :], iota_t[:],
                        dstloc_sb[:, col:col + 1], invdeg_sb[:, col:col + 1],
                        EQ, MUL)
                agg = psA.tile([P, D_H1], f32, tag="agg")
                for c in range(MC if "noagg" not in abl else 0):
                    nc.tensor.matmul(agg[:, 0:P], gath[:, c, :], oh1[:, c, :],
                                     start=(c == 0), stop=(c == MC - 1))
                meanT = mtp.tile([P, P], bf16, tag="meanT")
                nc.scalar.activation(meanT[:], agg[:, 0:P], COPY)

                xT = xT_res[:, b * P:(b + 1) * P]

                # node-major h1 (for the L2 src-side gathers)
                op = psO.tile([P, D_H1], f32, tag="outp")
                nc.tensor.matmul(op[:], meanT[:], w1l_sb[:],
                                 start=True, stop=False)
                nc.tensor.matmul(op[:], xT, w1r_sb[:],
                                 start=False, stop=False)
                nc.tensor.matmul(op[:], ones_t[:], b1_sb[:],
                                 start=False, stop=True)
                h1blk = hop.tile([P, D_H1], bf16, tag="hout")
                nc.scalar.activation(h1blk[:], op[:], RELU)
                nc.sync.dma_start(h1_own[b * P:(b + 1) * P, :], h1blk[:])
                # transposed h1 (for the L2 root term), SBUF-resident
                for s in range(2):
                    tph = psT.tile([P, P], f32, tag="tp")
                    nc.tensor.matmul(tph[:], w1l_sb[:, s * P:(s + 1) * P],
                                     meanT[:], start=True, stop=False)
                    nc.tensor.matmul(tph[:], w1r_sb[:, s * P:(s + 1) * P],
                                     xT, start=False, stop=True)
                    nc.scalar.activation(h1T_res[:, s, b * P:(b + 1) * P],
                                         tph[:], RELU,
                                         bias=b1t_sb[:, s:s + 1])

            # layer-2/3 weights first needed after L1
            w2l_sb = cp.tile([P, 2, D_H2], bf16, tag="w2l")
            nc.sync.dma_start(w2l_sb[:], w2l.rearrange("(s p) n -> p s n", p=P))
            w2r_sb = cp.tile([P, 2, D_H2], bf16, tag="w2r")
            nc.sync.dma_start(w2r_sb[:], w2r.rearrange("(s p) n -> p s n", p=P))
            b2t_sb = cp.tile([P, 8], f32, tag="b2t")
            nc.sync.dma_start(b2t_sb[:], b2t[:])
            w3lr_sb = cp.tile([P, 8, P], bf16, tag="w3lr")
            nc.sync.dma_start(w3lr_sb[:], w3lr.rearrange("(s p) n -> p s n", p=P))
            b3_sb = cp.tile([1, P], bf16, tag="b3")
            nc.sync.dma_start(b3_sb[:], b3pad[:])

            # ========== L2 src-side partial aggregation ==========
            # partial[dst, :] += invdeg[dst] * h1_own[srcloc, :] for local
            # edges, per dst block; one-hots built once, cached for L3.
            for gg in range((NBLK + GG - 1) // GG if "l2p" not in abl else 0):
                glo = gg * GG
                nblks = min(GG, NBLK - glo)
                gath = gp2.tile([P, GG * MC2, D_H1], bf16, tag="gath2")
                gather_block(gath, h1_own[:], gidx2_sb, glo * MC2,
                             nblks * MC2, D_H1, GSZ=GG * MC2)
                for j in range(nblks):
                    g = glo + j
                    for c in range(MC2):
                        col = g * MC2 + c
                        nc.vector.tensor_scalar(
                            ohc2[:, col, :], iota_t[:],
                            dstloc2_sb[:, col:col + 1],
                            invdeg2_sb[:, col:col + 1], EQ, MUL)
                    pp = psA.tile([P, D_H1], f32, tag="agg")
                    for c in range(MC2 if "noagg" not in abl else 0):
                        nc.tensor.matmul(pp[:], ohc2[:, g * MC2 + c, :],
                                         gath[:, j * MC2 + c, :],
                                         start=(c == 0), stop=(c == MC2 - 1))
                    pblk = hop.tile([P, D_H1], bf16, tag="pblk")
                    nc.scalar.activation(pblk[:], pp[:], COPY)
                    nc.sync.dma_start(part2[g * P:(g + 1) * P, :], pblk[:])

            if "noag" not in abl:
                nc.gpsimd.collective_compute(
                    "ReduceScatter", mybir.AluOpType.add,
                    replica_groups=[list(range(NCORES))],
                    ins=[part2.opt()], outs=[rs1out.opt()])

            # ====== L2 transform fused with L3 transform ======
            mean2_sb = cp.tile([P, NB, D_H1], bf16, tag="mean2")
            for b in range(NB):
                nc.sync.dma_start(mean2_sb[:, b, :],
                                  rs1out[b * P:(b + 1) * P, :])
            for b in range(NB if "l2t" not in abl else 0):
                # PE transpose of mean2 block -> feat-major
                meanT2 = mtp.tile([P, 2, P], bf16, tag="meanT2")
                for k in range(2):
                    pt = psT.tile([P, P], f32, tag="tp")
                    nc.tensor.matmul(pt[:],
                                     mean2_sb[:, b, k * P:(k + 1) * P],
                                     ident_t[:], start=True, stop=True)
                    nc.scalar.activation(meanT2[:, k, :], pt[:], COPY)
                # h2^T slices: mean half + root half, all in PSUM
                hT = htp.tile([P, 8, P], bf16, tag="hT")
                for s in range(8 if "noout" not in abl else 0):
                    hp = psT.tile([P, P], f32, tag="tp")
                    for k in range(2):
                        nc.tensor.matmul(
                            hp[:], w2l_sb[:, k, s * P:(s + 1) * P],
                            meanT2[:, k, :], start=(k == 0), stop=False)
                    for k in range(2):
                        nc.tensor.matmul(
                            hp[:], w2r_sb[:, k, s * P:(s + 1) * P],
                            h1T_res[:, k, b * P:(b + 1) * P],
                            start=False, stop=(k == 1))
                    nc.scalar.activation(hT[:, s, :], hp[:], RELU,
                                         bias=b2t_sb[:, s:s + 1])

                # [t3 | r3] = h2 @ [w3l | w3r] + [0 | b3] (lhsT = h2^T)
                trt = psO.tile([P, D_H1], f32, tag="outp")
                tr = trt[:, 0:P]
                for s in range(8):
                    nc.tensor.matmul(tr, hT[:, s, :], w3lr_sb[:, s, :],
                                     start=(s == 0), stop=False)
                nc.tensor.matmul(tr, ones_t[:], b3_sb[:],
                                 start=False, stop=True)
                t3blk = smp.tile([P, P], bf16, tag="t3blk")
                nc.vector.tensor_copy(t3blk[:, 0:D_OUT], trt[:, 0:D_OUT])
                nc.vector.memset(t3blk[:, D_OUT:P], 0.0)
                nc.vector.tensor_copy(r3_res[:, b, :], trt[:, D_OUT:P])
                nc.sync.dma_start(t3_own[b * P:(b + 1) * P, :], t3blk[:])

            # ========== L3 src-side partial aggregation ==========
            for gg in range((NBLK + GG - 1) // GG if "l3p" not in abl else 0):
                glo = gg * GG
                nblks = min(GG, NBLK - glo)
                gath = gp2.tile([P, GG * MC2, P], bf16, tag="gath3")
                gather_block(gath, t3_own[:], gidx2_sb, glo * MC2,
                             nblks * MC2, P, GSZ=GG * MC2)
                for j in range(nblks):
                    g = glo + j
                    pp = psA.tile([P, D_H1], f32, tag="agg")
                    for c in range(MC2 if "noagg" not in abl else 0):
                        nc.tensor.matmul(pp[:, 0:D_OUT], ohc2[:, g * MC2 + c, :],
                                         gath[:, j * MC2 + c, 0:D_OUT],
                                         start=(c == 0), stop=(c == MC2 - 1))
                    pblk = smp.tile([P, D_OUT], bf16, tag="pblk3")
                    nc.scalar.activation(pblk[:], pp[:, 0:D_OUT], COPY)
                    nc.sync.dma_start(part3[g * P:(g + 1) * P, :], pblk[:])

            if "noag" not in abl:
                nc.gpsimd.collective_compute(
                    "ReduceScatter", mybir.AluOpType.add,
                    replica_groups=[list(range(NCORES))],
                    ins=[part3.opt()], outs=[rs2out.opt()])

            # ================= log_softmax ==============
            mean3_sb = cp.tile([P, NB, D_OUT], bf16, tag="mean3")
            for b in range(NB):
                nc.sync.dma_start(mean3_sb[:, b, :],
                                  rs2out[b * P:(b + 1) * P, :])
            for b in range(NB if "l3t" not in abl else 0):
                nc.vector.tensor_tensor(y_res[:, b, :], mean3_sb[:, b, :],
                                        r3_res[:, b, :], ADD)
                nc.vector.tensor_reduce(negm_res[:, b:b + 1], y_res[:, b, :],
                                        AXX, MAX, negate=True)
                e = smp.tile([P, D_OUT], f32, tag="e")
                nc.scalar.activation(e[:], y_res[:, b, :], EXP,
                                     bias=negm_res[:, b:b + 1],
                                     scale=1.0, accum_out=ssum_res[:, b:b + 1])
            nc.scalar.activation(ls_res[:], ssum_res[:], LN)
            for b in range(NB if "l3t" not in abl else 0):
                ob = smp.tile([P, D_OUT], f32, tag="ob")
                nc.vector.tensor_scalar(ob[:], y_res[:, b, :],
                                        negm_res[:, b:b + 1],
                                        ls_res[:, b:b + 1], ADD, SUB)
                nc.sync.dma_start(outp[b * P:(b + 1) * P, :], ob[:])

    nc.compile()
    return nc


def _wrap16(a):
    """idx i -> partition i%16, col i//16; replicated to 128 partitions."""
    w = a.reshape(-1, 16).T
    return np.ascontiguousarray(np.tile(w, (8, 1)))


def _balanced_perm(deg):
    """Assign nodes to 80 blocks of 128 so block in-degree sums are even.

    Greedy: highest-degree node goes to the open block with the lowest
    degree sum. Returns newpos[old_node] -> permuted node id. This makes
    every block need exactly ceil(E/(NBLK*P)) = 16 edge chunks instead of
    the unbalanced max (17+), trimming gather + aggregation work ~6%.
    """
    import heapq
    order = np.argsort(-deg, kind="stable")
    heap = [(0, 0, g) for g in range(NBLK)]
    heapq.heapify(heap)
    newpos = np.empty(NPAD, np.int64)
    fill = np.zeros(NBLK, np.int64)
    for n in order:
        s, _, g = heapq.heappop(heap)
        newpos[n] = g * P + fill[g]
        fill[g] += 1
        if fill[g] < P:
            heapq.heappush(heap, (s + int(deg[n]), int(fill[g]), g))
    return newpos


def _prep(x, edge_index):
    src = np.asarray(edge_index[0], dtype=np.int64)
    dst = np.asarray(edge_index[1], dtype=np.int64)
    deg = np.bincount(dst, minlength=NPAD).astype(np.float64)
    invdeg_n = (1.0 / np.maximum(deg, 1.0)).astype(np.float32)

    newpos = _balanced_perm(deg)
    oldnode = np.empty(NPAD, np.int64)
    oldnode[newpos] = np.arange(NPAD)
    psrc = newpos[src]
    pdst = newpos[dst]

    # ---------- L1 dst-side organization ----------
    order = np.argsort(pdst, kind="stable")
    dsts = pdst[order]
    srcs = psrc[order]
    inv_e = invdeg_n[dst[order]]
    starts = np.searchsorted(dsts, np.arange(0, NPAD + P, P))
    cnt = starts[1:] - starts[:-1]
    MC = max(1, int(np.ceil(cnt.max() / P)))

    xp = np.zeros((NPAD, D_IN), dtype=np.float32)
    xp[:N_NODES] = x
    xp = xp[oldnode]           # permuted node order

    l1_per_core = []
    for r in range(NCORES):
        gparts, dparts, iparts = [], [], []
        for j in range(NB):
            g = r * NB + j
            lo, hi = starts[g], starts[g + 1]
            n = hi - lo
            o2 = lo + np.argsort(srcs[lo:hi], kind="stable")
            sg = np.zeros(MC * P, dtype=np.int16)
            dg = np.full(MC * P, -1.0, dtype=np.float32)
            ig = np.zeros(MC * P, dtype=np.float32)
            sg[:n] = srcs[o2].astype(np.int16)
            dg[:n] = (dsts[o2] - g * P).astype(np.float32)
            ig[:n] = inv_e[o2]
            gparts.append(_wrap16(sg))
            dparts.append(np.ascontiguousarray(dg.reshape(MC, P).T))
            iparts.append(np.ascontiguousarray(ig.reshape(MC, P).T))
        l1_per_core.append((
            np.concatenate(gparts, axis=1),
            np.concatenate(dparts, axis=1),
            np.concatenate(iparts, axis=1),
        ))

    # ---------- L2/L3 src-side organization ----------
    core_of_src = psrc // PER_CORE
    src_data = []   # per core: (srcloc, blk, dloc, iv) sorted by (blk, srcloc)
    MC2 = 1
    for r in range(NCORES):
        m = core_of_src == r
        es = psrc[m] - r * PER_CORE
        ed = pdst[m]
        iv = invdeg_n[dst[m]]
        blk = ed // P
        o2 = np.lexsort((es, blk))
        es, ed, iv, blk = es[o2], ed[o2], iv[o2], blk[o2]
        bstart = np.searchsorted(blk, np.arange(NBLK + 1))
        bcnt = bstart[1:] - bstart[:-1]
        MC2 = max(MC2, int(np.ceil(bcnt.max() / P)))
        src_data.append((es, ed, iv, blk, bstart))

    src_per_core = []
    for r in range(NCORES):
        es, ed, iv, blk, bstart = src_data[r]
        SL = MC2 * P
        sg = np.zeros(NBLK * SL, dtype=np.int16)
        dg = np.full(NBLK * SL, -1.0, dtype=np.float32)
        ig = np.zeros(NBLK * SL, dtype=np.float32)
        for g in range(NBLK):
            lo, hi = bstart[g], bstart[g + 1]
            n = hi - lo
            sg[g * SL:g * SL + n] = es[lo:hi].astype(np.int16)
            dg[g * SL:g * SL + n] = (ed[lo:hi] - g * P).astype(np.float32)
            ig[g * SL:g * SL + n] = iv[lo:hi]
        src_per_core.append((
            _wrap16(sg),
            np.ascontiguousarray(dg.reshape(NBLK * MC2, P).T),
            np.ascontiguousarray(ig.reshape(NBLK * MC2, P).T),
        ))

    return xp, l1_per_core, src_per_core, MC, MC2, newpos


def _make_in_maps(x, edge_index, w1l, w1r, b1, w2l, w2r, b2, w3l, w3r, b3):
    x = np.ascontiguousarray(np.asarray(x, dtype=np.float32))
    xp, l1_per_core, src_per_core, MC, MC2, newpos = _prep(
        x, np.asarray(edge_index))

    iota = np.tile(np.arange(P, dtype=np.float32), (P, 1))
    ident = np.eye(P, dtype=np.float32).astype(BF)
    b1v = np.asarray(b1, np.float32).reshape(-1)
    b2v = np.asarray(b2, np.float32).reshape(-1)
    xbf = xp.astype(BF)
    common = {
        "xbf": xbf,
        "w1l": np.asarray(w1l, np.float32).astype(BF),
        "w1r": np.asarray(w1r, np.float32).astype(BF),
        "b1": b1v.reshape(1, D_H1).astype(BF),
        "b1t": np.ascontiguousarray(b1v.reshape(2, P).T),
        "w2l": np.asarray(w2l, np.float32).astype(BF),
        "w2r": np.asarray(w2r, np.float32).astype(BF),
        "b2t": np.ascontiguousarray(b2v.reshape(8, P).T),
        "w3lr": np.ascontiguousarray(np.concatenate(
            [np.asarray(w3l, np.float32), np.asarray(w3r, np.float32)],
            axis=1)).astype(BF),
        "b3pad": np.concatenate(
            [np.zeros(D_OUT, np.float32),
             np.asarray(b3, np.float32).reshape(-1)]).reshape(1, P).astype(BF),
        "iota_in": iota,
        "ident_in": ident,
    }
    in_maps = []
    for r in range(NCORES):
        g1, d1, i1 = l1_per_core[r]
        g2, d2, i2 = src_per_core[r]
        m = dict(common)
        m["xownT"] = np.ascontiguousarray(
            xbf[r * PER_CORE:(r + 1) * PER_CORE].T)
        m["gidx"] = g1
        m["dstloc"] = d1
        m["invdeg"] = i1
        m["gidx2"] = g2
        m["dstloc2"] = d2
        m["invdeg2"] = i2
        in_maps.append(m)
    return in_maps, (MC, MC2), newpos


def kernel(x, edge_index, w1l, w1r, b1, w2l, w2r, b2, w3l, w3r, b3):
    global LAST_RESULTS
    import os
    from concourse.bass_utils import run_bass_kernel_spmd

    if os.environ.get("BASS_TRACE"):
        try:
            import antenv.axon_hooks  # noqa: F401
        except ImportError:
            os.environ.pop("BASS_TRACE", None)  # no NTFF hook here

    in_maps, key, newpos = _make_in_maps(x, edge_index, w1l, w1r, b1, w2l,
                                         w2r, b2, w3l, w3r, b3)
    if key not in _CACHE:
        _CACHE[key] = _build(key)
    nc = _CACHE[key]

    res = run_bass_kernel_spmd(nc, in_maps, core_ids=list(range(NCORES)))
    LAST_RESULTS = res
    out = np.concatenate([res.results[r]["out"] for r in range(NCORES)], axis=0)
    return np.ascontiguousarray(out[newpos[:N_NODES]])


# revision 5
# speedup vs baseline: 1.4738x; 1.2934x over previous
"""3-layer GraphSAGE (mean aggr) on Trainium2, 8-core SPMD, fp16 compute.

Strategy (graph/data parallel, ReduceScatter formulation):
  - Nodes padded 10000 -> 10240, degree-balanced permutation into 80
    blocks of 128 (block in-degrees ~equal so L1 needs exactly 16 edge
    chunks per block). Core r owns blocks [10r, 10r+10).
  - L1 (x replicated, no exchange): dst-side gather of x rows per owned
    block + one-hot matmul segment-mean on the PE; produces node-major
    h1_own (DRAM, for L2 gathers) and feat-major h1T (SBUF, for the L2
    root term).
  - L2/L3 use SRC-side partial aggregation + ReduceScatter instead of
    AllGather: core r takes the ~20k edges whose SOURCE it owns, grouped
    by dst block (80 groups x MC2 chunks of 128, padded), gathers its
    own h1 rows per edge, and accumulates per-dst-block partial sums
    (one-hot matmuls, invdeg folded in) into a [10240,256] fp16 partial
    tensor. One ReduceScatter(add) then gives each core the exact mean
    rows for its own 1280 nodes.  Collective cost drops ~5x vs AllGather
    (charged on the 0.66MB scattered output, not the 5.2MB gathered one).
  - L2 transform consumes the scattered mean directly (PE transpose via
    identity matmul to feat-major), folds the root half as extra
    accumulating matmuls from SBUF-resident h1T, and emits h2 transposed
    so t3|r3 = h2 @ [w3l|w3r] runs in the same block loop (h2 never in
    DRAM).  L3 repeats the src-side partial + ReduceScatter on the
    64-wide t3 (padded to 128 cols for the 256B-min gather row size).
  - log_softmax: all EXP passes first (accumulating per-block sums),
    then ONE Ln over [128,10] — avoids per-block activation-table
    thrash.  PSUM accumulation is f32 end-to-end.
"""

import numpy as np
BF = np.float16

N_NODES = 10000
NPAD = 10240
NCORES = 8
P = 128
NB = 10                      # dst blocks per core
NBLK = NPAD // P             # 80
PER_CORE = NB * P            # 1280
D_IN, D_H1, D_H2, D_OUT = 128, 256, 1024, 64

_CACHE = {}
LAST_RESULTS = None          # test harness reads exec_time_ns from here


def _build(key):
    import os
    import concourse.bacc as bacc
    import concourse.mybir as mybir
    import concourse.tile as tile

    MC, MC2 = key
    abl = set(os.environ.get("KABL", "").split(","))
    GG = int(os.environ.get("KGG", "8"))      # dst blocks per src-side gather

    f32 = mybir.dt.float32
    bf16 = mybir.dt.float16
    i16 = mybir.dt.int16
    nc = bacc.Bacc("TRN2", target_bir_lowering=False, debug=False,
                   num_devices=NCORES)

    xbf = nc.dram_tensor("xbf", [NPAD, D_IN], bf16, kind="ExternalInput")
    xownT = nc.dram_tensor("xownT", [P, PER_CORE], bf16, kind="ExternalInput")
    w1l = nc.dram_tensor("w1l", [D_IN, D_H1], bf16, kind="ExternalInput")
    w1r = nc.dram_tensor("w1r", [D_IN, D_H1], bf16, kind="ExternalInput")
    b1 = nc.dram_tensor("b1", [1, D_H1], bf16, kind="ExternalInput")
    b1t = nc.dram_tensor("b1t", [P, 2], f32, kind="ExternalInput")
    w2l = nc.dram_tensor("w2l", [D_H1, D_H2], bf16, kind="ExternalInput")
    w2r = nc.dram_tensor("w2r", [D_H1, D_H2], bf16, kind="ExternalInput")
    b2t = nc.dram_tensor("b2t", [P, 8], f32, kind="ExternalInput")
    w3lr = nc.dram_tensor("w3lr", [D_H2, P], bf16, kind="ExternalInput")
    b3pad = nc.dram_tensor("b3pad", [1, P], bf16, kind="ExternalInput")
    gidx = nc.dram_tensor("gidx", [P, NB * MC * 8], i16, kind="ExternalInput")
    dstloc = nc.dram_tensor("dstloc", [P, NB * MC], f32, kind="ExternalInput")
    invdeg = nc.dram_tensor("invdeg", [P, NB * MC], f32, kind="ExternalInput")
    gidx2 = nc.dram_tensor("gidx2", [P, NBLK * MC2 * 8], i16,
                           kind="ExternalInput")
    dstloc2 = nc.dram_tensor("dstloc2", [P, NBLK * MC2], f32,
                             kind="ExternalInput")
    invdeg2 = nc.dram_tensor("invdeg2", [P, NBLK * MC2], f32,
                             kind="ExternalInput")
    iota_in = nc.dram_tensor("iota_in", [P, P], f32, kind="ExternalInput")
    ident_in = nc.dram_tensor("ident_in", [P, P], bf16, kind="ExternalInput")
    outp = nc.dram_tensor("out", [PER_CORE, D_OUT], f32, kind="ExternalOutput")

    EXP = mybir.ActivationFunctionType.Exp
    LN = mybir.ActivationFunctionType.Ln
    RELU = mybir.ActivationFunctionType.Relu
    COPY = mybir.ActivationFunctionType.Copy
    EQ = mybir.AluOpType.is_equal
    MUL = mybir.AluOpType.mult
    SUB = mybir.AluOpType.subtract
    ADD = mybir.AluOpType.add
    MAX = mybir.AluOpType.max
    AXX = mybir.AxisListType.X

    with tile.TileContext(nc) as tc:
        with (
            tc.tile_pool(name="const", bufs=1) as cp,
            tc.tile_pool(name="gath", bufs=3) as gp,
            tc.tile_pool(name="gath2", bufs=2) as gp2,
            tc.tile_pool(name="oh1", bufs=4) as ohp,
            tc.tile_pool(name="ht", bufs=2) as htp,
            tc.tile_pool(name="meant", bufs=3) as mtp,
            tc.tile_pool(name="hout", bufs=3) as hop,
            tc.tile_pool(name="small", bufs=6) as smp,
            tc.tile_pool(name="psA", bufs=3, space="PSUM") as psA,
            tc.tile_pool(name="psO", bufs=2, space="PSUM") as psO,
            tc.tile_pool(name="psT", bufs=3, space="PSUM") as psT,
            tc.tile_pool(name="dram", bufs=1, space="DRAM") as dram,
        ):
            # ---- constants ----
            iota_t = cp.tile([P, P], f32, tag="iota")
            nc.sync.dma_start(iota_t[:], iota_in[:])
            ident_t = cp.tile([P, P], bf16, tag="ident")
            nc.sync.dma_start(ident_t[:], ident_in[:])
            ones_t = cp.tile([1, P], bf16, tag="ones")
            nc.vector.memset(ones_t[:], 1.0)
            gidx_sb = cp.tile([P, NB * MC * 8], i16, tag="gidx")
            nc.sync.dma_start(gidx_sb[:], gidx[:])
            dstloc_sb = cp.tile([P, NB * MC], f32, tag="dstloc")
            nc.sync.dma_start(dstloc_sb[:], dstloc[:])
            invdeg_sb = cp.tile([P, NB * MC], f32, tag="invdeg")
            nc.sync.dma_start(invdeg_sb[:], invdeg[:])
            gidx2_sb = cp.tile([P, NBLK * MC2 * 8], i16, tag="gidx2")
            nc.sync.dma_start(gidx2_sb[:], gidx2[:])
            dstloc2_sb = cp.tile([P, NBLK * MC2], f32, tag="dstloc2")
            nc.sync.dma_start(dstloc2_sb[:], dstloc2[:])
            invdeg2_sb = cp.tile([P, NBLK * MC2], f32, tag="invdeg2")
            nc.sync.dma_start(invdeg2_sb[:], invdeg2[:])

            w1l_sb = cp.tile([P, D_H1], bf16, tag="w1l")
            nc.sync.dma_start(w1l_sb[:], w1l[:])
            w1r_sb = cp.tile([P, D_H1], bf16, tag="w1r")
            nc.sync.dma_start(w1r_sb[:], w1r[:])
            b1_sb = cp.tile([1, D_H1], bf16, tag="b1")
            nc.sync.dma_start(b1_sb[:], b1[:])
            b1t_sb = cp.tile([P, 2], f32, tag="b1t")
            nc.sync.dma_start(b1t_sb[:], b1t[:])

            # resident cross-phase SBUF state
            xT_res = cp.tile([P, PER_CORE], bf16, tag="xT")
            nc.sync.dma_start(xT_res[:], xownT[:])
            h1T_res = cp.tile([P, 2, PER_CORE], bf16, tag="h1T")
            h1st = cp.tile([P, NB, D_H1], bf16, tag="h1st")
            t3st = cp.tile([P, NB, P], bf16, tag="t3st")
            y_out = cp.tile([P, NB, D_OUT], f32, tag="yout")
            r3_res = cp.tile([P, NB, D_OUT], f32, tag="r3")
            y_res = cp.tile([P, NB, D_OUT], f32, tag="y")
            negm_res = cp.tile([P, NB], f32, tag="negm")
            ssum_res = cp.tile([P, NB], f32, tag="ssum")
            ls_res = cp.tile([P, NB], f32, tag="ls")
            ohc2 = cp.tile([P, NBLK * MC2, P], bf16, tag="ohc2")

            # ---- DRAM intermediates ----
            h1_own = dram.tile([PER_CORE, D_H1], bf16, tag="h1o")
            t3_own = dram.tile([PER_CORE, P], bf16, tag="t3o")
            part2 = dram.tile([NPAD, D_H1], bf16, tag="p2")
            rs1out = dram.tile([PER_CORE, D_H1], bf16, tag="rs1")
            part3 = dram.tile([NPAD, D_OUT], bf16, tag="p3")
            rs2out = dram.tile([PER_CORE, D_OUT], bf16, tag="rs2")

            def gather_block(dst_tile, src_ap, idx_sb, c0, nchunk, elem,
                             GSZ=8):
                """Gather nchunk*128 rows starting at chunk c0 of idx_sb."""
                if "nogather" in abl:
                    return
                g0 = 0
                while g0 < nchunk:
                    gsz = min(GSZ, nchunk - g0)
                    cc = (c0 + g0) * 8
                    nc.gpsimd.dma_gather(
                        dst_tile[:, g0:g0 + gsz, :], src_ap,
                        idx_sb[:, cc:cc + gsz * 8],
                        gsz * P, gsz * P, elem, single_packet=False)
                    g0 += gsz

            # ================= Layer 1 (dst-side, local) =================
            for b in range(NB if "l1" not in abl else 0):
                gath = gp.tile([P, MC, D_IN], bf16, tag="gath")
                gather_block(gath, xbf[:], gidx_sb, b * MC, MC, D_IN)
                oh1 = ohp.tile([P, MC, P], bf16, tag="oh1")
                for c in range(MC):
                    col = b * MC + c
                    nc.vector.tensor_scalar(
                        oh1[:, c, :], iota_t[:],
                        dstloc_sb[:, col:col + 1], invdeg_sb[:, col:col + 1],
                        EQ, MUL)
                agg = psA.tile([P, D_H1], f32, tag="agg")
                for c in range(MC if "noagg" not in abl else 0):
                    nc.tensor.matmul(agg[:, 0:P], gath[:, c, :], oh1[:, c, :],
                                     start=(c == 0), stop=(c == MC - 1))
                meanT = mtp.tile([P, P], bf16, tag="meanT")
                nc.scalar.activation(meanT[:], agg[:, 0:P], COPY)

                xT = xT_res[:, b * P:(b + 1) * P]

                # node-major h1 (for the L2 src-side gathers)
                op = psO.tile([P, D_H1], f32, tag="outp")
                nc.tensor.matmul(op[:], meanT[:], w1l_sb[:],
                                 start=True, stop=False)
                nc.tensor.matmul(op[:], xT, w1r_sb[:],
                                 start=False, stop=False)
                nc.tensor.matmul(op[:], ones_t[:], b1_sb[:],
                                 start=False, stop=True)
                nc.scalar.activation(h1st[:, b, :], op[:], RELU)
                # transposed h1 (for the L2 root term), SBUF-resident
                for s in range(2):
                    tph = psT.tile([P, P], f32, tag="tp")
                    nc.tensor.matmul(tph[:], w1l_sb[:, s * P:(s + 1) * P],
                                     meanT[:], start=True, stop=False)
                    nc.tensor.matmul(tph[:], w1r_sb[:, s * P:(s + 1) * P],
                                     xT, start=False, stop=True)
                    nc.scalar.activation(h1T_res[:, s, b * P:(b + 1) * P],
                                         tph[:], RELU,
                                         bias=b1t_sb[:, s:s + 1])

            nc.sync.dma_start(
                h1_own[:].rearrange("(b p) n -> p b n", p=P), h1st[:])

            # src-side one-hots for L2/L3: only need dstloc2/invdeg2, so
            # emit them here to overlap the L1 tail / L2 gather ramp on DVE
            for col in range(NBLK * MC2):
                nc.vector.tensor_scalar(
                    ohc2[:, col, :], iota_t[:],
                    dstloc2_sb[:, col:col + 1],
                    invdeg2_sb[:, col:col + 1], EQ, MUL)

            # layer-2/3 weights first needed after L1
            w2l_sb = cp.tile([P, 2, D_H2], bf16, tag="w2l")
            nc.sync.dma_start(w2l_sb[:], w2l.rearrange("(s p) n -> p s n", p=P))
            w2r_sb = cp.tile([P, 2, D_H2], bf16, tag="w2r")
            nc.sync.dma_start(w2r_sb[:], w2r.rearrange("(s p) n -> p s n", p=P))
            b2t_sb = cp.tile([P, 8], f32, tag="b2t")
            nc.sync.dma_start(b2t_sb[:], b2t[:])
            w3lr_sb = cp.tile([P, 8, P], bf16, tag="w3lr")
            nc.sync.dma_start(w3lr_sb[:], w3lr.rearrange("(s p) n -> p s n", p=P))
            b3_sb = cp.tile([1, P], bf16, tag="b3")
            nc.sync.dma_start(b3_sb[:], b3pad[:])

            # ========== L2 src-side partial aggregation ==========
            # partial[dst, :] += invdeg[dst] * h1_own[srcloc, :] for local
            # edges, per dst block; one-hots built once, cached for L3.
            for gg in range((NBLK + GG - 1) // GG if "l2p" not in abl else 0):
                glo = gg * GG
                nblks = min(GG, NBLK - glo)
                gath = gp2.tile([P, GG * MC2, D_H1], bf16, tag="gath2")
                gather_block(gath, h1_own[:], gidx2_sb, glo * MC2,
                             nblks * MC2, D_H1, GSZ=GG * MC2)
                st2 = hop.tile([P, GG, D_H1], bf16, tag="st2")
                for j in range(nblks):
                    g = glo + j
                    pp = psA.tile([P, D_H1], f32, tag="agg")
                    for c in range(MC2 if "noagg" not in abl else 0):
                        nc.tensor.matmul(pp[:], ohc2[:, g * MC2 + c, :],
                                         gath[:, j * MC2 + c, :],
                                         start=(c == 0), stop=(c == MC2 - 1))
                    nc.scalar.activation(st2[:, j, :], pp[:], COPY)
                nc.sync.dma_start(
                    part2[glo * P:(glo + nblks) * P, :].rearrange(
                        "(j p) n -> p j n", p=P), st2[:])

            if "noag" not in abl:
                nc.gpsimd.collective_compute(
                    "ReduceScatter", mybir.AluOpType.add,
                    replica_groups=[list(range(NCORES))],
                    ins=[part2.opt()], outs=[rs1out.opt()])

            # ====== L2 transform fused with L3 transform ======
            nc.vector.memset(t3st[:], 0.0)
            mean2_sb = cp.tile([P, NB, D_H1], bf16, tag="mean2")
            nc.sync.dma_start(
                mean2_sb[:], rs1out[:].rearrange("(b p) n -> p b n", p=P))
            for b in range(NB if "l2t" not in abl else 0):
                # PE transpose of mean2 block -> feat-major
                meanT2 = mtp.tile([P, 2, P], bf16, tag="meanT2")
                for k in range(2):
                    pt = psT.tile([P, P], f32, tag="tp")
                    nc.tensor.matmul(pt[:],
                                     mean2_sb[:, b, k * P:(k + 1) * P],
                                     ident_t[:], start=True, stop=True)
                    nc.scalar.activation(meanT2[:, k, :], pt[:], COPY)
                # h2^T slices: mean half + root half, all in PSUM
                hT = htp.tile([P, 8, P], bf16, tag="hT")
                for s in range(8 if "noout" not in abl else 0):
                    hp = psT.tile([P, P], f32, tag="tp")
                    for k in range(2):
                        nc.tensor.matmul(
                            hp[:], w2l_sb[:, k, s * P:(s + 1) * P],
                            meanT2[:, k, :], start=(k == 0), stop=False)
                    for k in range(2):
                        nc.tensor.matmul(
                            hp[:], w2r_sb[:, k, s * P:(s + 1) * P],
                            h1T_res[:, k, b * P:(b + 1) * P],
                            start=False, stop=(k == 1))
                    nc.scalar.activation(hT[:, s, :], hp[:], RELU,
                                         bias=b2t_sb[:, s:s + 1])

                # [t3 | r3] = h2 @ [w3l | w3r] + [0 | b3] (lhsT = h2^T)
                trt = psO.tile([P, D_H1], f32, tag="outp")
                tr = trt[:, 0:P]
                for s in range(8):
                    nc.tensor.matmul(tr, hT[:, s, :], w3lr_sb[:, s, :],
                                     start=(s == 0), stop=False)
                nc.tensor.matmul(tr, ones_t[:], b3_sb[:],
                                 start=False, stop=True)
                nc.vector.tensor_copy(t3st[:, b, 0:D_OUT], trt[:, 0:D_OUT])
                nc.vector.tensor_copy(r3_res[:, b, :], trt[:, D_OUT:P])

            nc.sync.dma_start(
                t3_own[:].rearrange("(b p) n -> p b n", p=P), t3st[:])

            # ========== L3 src-side partial aggregation ==========
            for gg in range((NBLK + GG - 1) // GG if "l3p" not in abl else 0):
                glo = gg * GG
                nblks = min(GG, NBLK - glo)
                gath = gp2.tile([P, GG * MC2, P], bf16, tag="gath3")
                gather_block(gath, t3_own[:], gidx2_sb, glo * MC2,
                             nblks * MC2, P, GSZ=GG * MC2)
                st3 = hop.tile([P, GG, D_OUT], bf16, tag="st3")
                for j in range(nblks):
                    g = glo + j
                    pp = psA.tile([P, D_H1], f32, tag="agg")
                    for c in range(MC2 if "noagg" not in abl else 0):
                        nc.tensor.matmul(pp[:, 0:D_OUT], ohc2[:, g * MC2 + c, :],
                                         gath[:, j * MC2 + c, 0:D_OUT],
                                         start=(c == 0), stop=(c == MC2 - 1))
                    nc.scalar.activation(st3[:, j, :], pp[:, 0:D_OUT], COPY)
                nc.sync.dma_start(
                    part3[glo * P:(glo + nblks) * P, :].rearrange(
                        "(j p) n -> p j n", p=P), st3[:])

            if "noag" not in abl:
                nc.gpsimd.collective_compute(
                    "ReduceScatter", mybir.AluOpType.add,
                    replica_groups=[list(range(NCORES))],
                    ins=[part3.opt()], outs=[rs2out.opt()])

            # ================= log_softmax ==============
            mean3_sb = cp.tile([P, NB, D_OUT], bf16, tag="mean3")
            nc.sync.dma_start(
                mean3_sb[:], rs2out[:].rearrange("(b p) n -> p b n", p=P))
            for b in range(NB if "l3t" not in abl else 0):
                nc.vector.tensor_tensor(y_res[:, b, :], mean3_sb[:, b, :],
                                        r3_res[:, b, :], ADD)
                nc.vector.tensor_reduce(negm_res[:, b:b + 1], y_res[:, b, :],
                                        AXX, MAX, negate=True)
                e = smp.tile([P, D_OUT], f32, tag="e")
                nc.scalar.activation(e[:], y_res[:, b, :], EXP,
                                     bias=negm_res[:, b:b + 1],
                                     scale=1.0, accum_out=ssum_res[:, b:b + 1])
            nc.scalar.activation(ls_res[:], ssum_res[:], LN)
            for b in range(NB if "l3t" not in abl else 0):
                nc.vector.tensor_scalar(y_out[:, b, :], y_res[:, b, :],
                                        negm_res[:, b:b + 1],
                                        ls_res[:, b:b + 1], ADD, SUB)
            nc.sync.dma_start(
                outp[:].rearrange("(b p) n -> p b n", p=P), y_out[:])

    nc.compile()
    return nc


def _wrap16(a):
    """idx i -> partition i%16, col i//16; replicated to 128 partitions."""
    w = a.reshape(-1, 16).T
    return np.ascontiguousarray(np.tile(w, (8, 1)))


def _balanced_perm(deg):
    """Assign nodes to 80 blocks of 128 so block in-degree sums are even.

    Greedy: highest-degree node goes to the open block with the lowest
    degree sum. Returns newpos[old_node] -> permuted node id. This makes
    every block need exactly ceil(E/(NBLK*P)) = 16 edge chunks instead of
    the unbalanced max (17+), trimming gather + aggregation work ~6%.
    """
    import heapq
    order = np.argsort(-deg, kind="stable")
    heap = [(0, 0, g) for g in range(NBLK)]
    heapq.heapify(heap)
    newpos = np.empty(NPAD, np.int64)
    fill = np.zeros(NBLK, np.int64)
    for n in order:
        s, _, g = heapq.heappop(heap)
        newpos[n] = g * P + fill[g]
        fill[g] += 1
        if fill[g] < P:
            heapq.heappush(heap, (s + int(deg[n]), int(fill[g]), g))
    return newpos


def _prep(x, edge_index):
    src = np.asarray(edge_index[0], dtype=np.int64)
    dst = np.asarray(edge_index[1], dtype=np.int64)
    deg = np.bincount(dst, minlength=NPAD).astype(np.float64)
    invdeg_n = (1.0 / np.maximum(deg, 1.0)).astype(np.float32)

    newpos = _balanced_perm(deg)
    oldnode = np.empty(NPAD, np.int64)
    oldnode[newpos] = np.arange(NPAD)
    psrc = newpos[src]
    pdst = newpos[dst]

    # ---------- L1 dst-side organization ----------
    order = np.argsort(pdst, kind="stable")
    dsts = pdst[order]
    srcs = psrc[order]
    inv_e = invdeg_n[dst[order]]
    starts = np.searchsorted(dsts, np.arange(0, NPAD + P, P))
    cnt = starts[1:] - starts[:-1]
    MC = max(1, int(np.ceil(cnt.max() / P)))

    xp = np.zeros((NPAD, D_IN), dtype=np.float32)
    xp[:N_NODES] = x
    xp = xp[oldnode]           # permuted node order

    l1_per_core = []
    for r in range(NCORES):
        gparts, dparts, iparts = [], [], []
        for j in range(NB):
            g = r * NB + j
            lo, hi = starts[g], starts[g + 1]
            n = hi - lo
            o2 = lo + np.argsort(srcs[lo:hi], kind="stable")
            sg = np.zeros(MC * P, dtype=np.int16)
            dg = np.full(MC * P, -1.0, dtype=np.float32)
            ig = np.zeros(MC * P, dtype=np.float32)
            sg[:n] = srcs[o2].astype(np.int16)
            dg[:n] = (dsts[o2] - g * P).astype(np.float32)
            ig[:n] = inv_e[o2]
            gparts.append(_wrap16(sg))
            dparts.append(np.ascontiguousarray(dg.reshape(MC, P).T))
            iparts.append(np.ascontiguousarray(ig.reshape(MC, P).T))
        l1_per_core.append((
            np.concatenate(gparts, axis=1),
            np.concatenate(dparts, axis=1),
            np.concatenate(iparts, axis=1),
        ))

    # ---------- L2/L3 src-side organization ----------
    core_of_src = psrc // PER_CORE
    src_data = []   # per core: (srcloc, blk, dloc, iv) sorted by (blk, srcloc)
    MC2 = 1
    for r in range(NCORES):
        m = core_of_src == r
        es = psrc[m] - r * PER_CORE
        ed = pdst[m]
        iv = invdeg_n[dst[m]]
        blk = ed // P
        o2 = np.lexsort((es, blk))
        es, ed, iv, blk = es[o2], ed[o2], iv[o2], blk[o2]
        bstart = np.searchsorted(blk, np.arange(NBLK + 1))
        bcnt = bstart[1:] - bstart[:-1]
        MC2 = max(MC2, int(np.ceil(bcnt.max() / P)))
        src_data.append((es, ed, iv, blk, bstart))

    src_per_core = []
    for r in range(NCORES):
        es, ed, iv, blk, bstart = src_data[r]
        SL = MC2 * P
        sg = np.zeros(NBLK * SL, dtype=np.int16)
        dg = np.full(NBLK * SL, -1.0, dtype=np.float32)
        ig = np.zeros(NBLK * SL, dtype=np.float32)
        for g in range(NBLK):
            lo, hi = bstart[g], bstart[g + 1]
            n = hi - lo
            sg[g * SL:g * SL + n] = es[lo:hi].astype(np.int16)
            dg[g * SL:g * SL + n] = (ed[lo:hi] - g * P).astype(np.float32)
            ig[g * SL:g * SL + n] = iv[lo:hi]
        src_per_core.append((
            _wrap16(sg),
            np.ascontiguousarray(dg.reshape(NBLK * MC2, P).T),
            np.ascontiguousarray(ig.reshape(NBLK * MC2, P).T),
        ))

    return xp, l1_per_core, src_per_core, MC, MC2, newpos


def _make_in_maps(x, edge_index, w1l, w1r, b1, w2l, w2r, b2, w3l, w3r, b3):
    x = np.ascontiguousarray(np.asarray(x, dtype=np.float32))
    xp, l1_per_core, src_per_core, MC, MC2, newpos = _prep(
        x, np.asarray(edge_index))

    iota = np.tile(np.arange(P, dtype=np.float32), (P, 1))
    ident = np.eye(P, dtype=np.float32).astype(BF)
    b1v = np.asarray(b1, np.float32).reshape(-1)
    b2v = np.asarray(b2, np.float32).reshape(-1)
    xbf = xp.astype(BF)
    common = {
        "xbf": xbf,
        "w1l": np.asarray(w1l, np.float32).astype(BF),
        "w1r": np.asarray(w1r, np.float32).astype(BF),
        "b1": b1v.reshape(1, D_H1).astype(BF),
        "b1t": np.ascontiguousarray(b1v.reshape(2, P).T),
        "w2l": np.asarray(w2l, np.float32).astype(BF),
        "w2r": np.asarray(w2r, np.float32).astype(BF),
        "b2t": np.ascontiguousarray(b2v.reshape(8, P).T),
        "w3lr": np.ascontiguousarray(np.concatenate(
            [np.asarray(w3l, np.float32), np.asarray(w3r, np.float32)],
            axis=1)).astype(BF),
        "b3pad": np.concatenate(
            [np.zeros(D_OUT, np.float32),
             np.asarray(b3, np.float32).reshape(-1)]).reshape(1, P).astype(BF),
        "iota_in": iota,
        "ident_in": ident,
    }
    in_maps = []
    for r in range(NCORES):
        g1, d1, i1 = l1_per_core[r]
        g2, d2, i2 = src_per_core[r]
        m = dict(common)
        m["xownT"] = np.ascontiguousarray(
            xbf[r * PER_CORE:(r + 1) * PER_CORE].T)
        m["gidx"] = g1
        m["dstloc"] = d1
        m["invdeg"] = i1
        m["gidx2"] = g2
        m["dstloc2"] = d2
        m["invdeg2"] = i2
        in_maps.append(m)
    return in_maps, (MC, MC2), newpos


def kernel(x, edge_index, w1l, w1r, b1, w2l, w2r, b2, w3l, w3r, b3):
    global LAST_RESULTS
    import os
    from concourse.bass_utils import run_bass_kernel_spmd

    if os.environ.get("BASS_TRACE"):
        try:
            import antenv.axon_hooks  # noqa: F401
        except ImportError:
            os.environ.pop("BASS_TRACE", None)  # no NTFF hook here

    in_maps, key, newpos = _make_in_maps(x, edge_index, w1l, w1r, b1, w2l,
                                         w2r, b2, w3l, w3r, b3)
    if key not in _CACHE:
        _CACHE[key] = _build(key)
    nc = _CACHE[key]

    res = run_bass_kernel_spmd(nc, in_maps, core_ids=list(range(NCORES)))
    LAST_RESULTS = res
    out = np.concatenate([res.results[r]["out"] for r in range(NCORES)], axis=0)
    return np.ascontiguousarray(out[newpos[:N_NODES]])


# revision 6
# speedup vs baseline: 1.4798x; 1.0040x over previous
"""3-layer GraphSAGE (mean aggr) on Trainium2, 8-core SPMD, fp16 compute.

Strategy (graph/data parallel, ReduceScatter formulation):
  - Nodes padded 10000 -> 10240, degree-balanced permutation into 80
    blocks of 128 (block in-degrees ~equal so L1 needs exactly 16 edge
    chunks per block). Core r owns blocks [10r, 10r+10).
  - L1 (x replicated, no exchange): dst-side gather of x rows per owned
    block + one-hot matmul segment-mean on the PE; produces node-major
    h1_own (DRAM, for L2 gathers) and feat-major h1T (SBUF, for the L2
    root term).
  - L2/L3 use SRC-side partial aggregation + ReduceScatter instead of
    AllGather: core r takes the ~20k edges whose SOURCE it owns, grouped
    by dst block (80 groups x MC2 chunks of 128, padded), gathers its
    own h1 rows per edge, and accumulates per-dst-block partial sums
    (one-hot matmuls, invdeg folded in) into a [10240,256] fp16 partial
    tensor. One ReduceScatter(add) then gives each core the exact mean
    rows for its own 1280 nodes.  Collective cost drops ~5x vs AllGather
    (charged on the 0.66MB scattered output, not the 5.2MB gathered one).
  - L2 transform consumes the scattered mean directly (PE transpose via
    identity matmul to feat-major), folds the root half as extra
    accumulating matmuls from SBUF-resident h1T, and emits h2 transposed
    so t3|r3 = h2 @ [w3l|w3r] runs in the same block loop (h2 never in
    DRAM).  L3 repeats the src-side partial + ReduceScatter on the
    64-wide t3 (padded to 128 cols for the 256B-min gather row size).
  - log_softmax: all EXP passes first (accumulating per-block sums),
    then ONE Ln over [128,10] — avoids per-block activation-table
    thrash.  PSUM accumulation is f32 end-to-end.
"""

import numpy as np
BF = np.float16

N_NODES = 10000
NPAD = 10240
NCORES = 8
P = 128
NB = 10                      # dst blocks per core
NBLK = NPAD // P             # 80
PER_CORE = NB * P            # 1280
D_IN, D_H1, D_H2, D_OUT = 128, 256, 1024, 64

_CACHE = {}
LAST_RESULTS = None          # test harness reads exec_time_ns from here


def _build(key):
    import os
    import concourse.bacc as bacc
    import concourse.mybir as mybir
    import concourse.tile as tile

    MC, MC2 = key
    abl = set(os.environ.get("KABL", "").split(","))
    GG = int(os.environ.get("KGG", "8"))      # dst blocks per src-side gather

    f32 = mybir.dt.float32
    bf16 = mybir.dt.float16
    i16 = mybir.dt.int16
    nc = bacc.Bacc("TRN2", target_bir_lowering=False, debug=False,
                   num_devices=NCORES)

    xbf = nc.dram_tensor("xbf", [NPAD, D_IN], bf16, kind="ExternalInput")
    xownT = nc.dram_tensor("xownT", [P, PER_CORE], bf16, kind="ExternalInput")
    w1l = nc.dram_tensor("w1l", [D_IN, D_H1], bf16, kind="ExternalInput")
    w1r = nc.dram_tensor("w1r", [D_IN, D_H1], bf16, kind="ExternalInput")
    b1 = nc.dram_tensor("b1", [1, D_H1], bf16, kind="ExternalInput")
    b1t = nc.dram_tensor("b1t", [P, 2], f32, kind="ExternalInput")
    w2l = nc.dram_tensor("w2l", [D_H1, D_H2], bf16, kind="ExternalInput")
    w2r = nc.dram_tensor("w2r", [D_H1, D_H2], bf16, kind="ExternalInput")
    b2row = nc.dram_tensor("b2row", [1, D_H2], bf16, kind="ExternalInput")
    w3lr = nc.dram_tensor("w3lr", [D_H2, P], bf16, kind="ExternalInput")
    b3pad = nc.dram_tensor("b3pad", [1, P], bf16, kind="ExternalInput")
    gidx = nc.dram_tensor("gidx", [P, NB * MC * 8], i16, kind="ExternalInput")
    dstloc = nc.dram_tensor("dstloc", [P, NB * MC], f32, kind="ExternalInput")
    invdeg = nc.dram_tensor("invdeg", [P, NB * MC], f32, kind="ExternalInput")
    gidx2 = nc.dram_tensor("gidx2", [P, NBLK * MC2 * 8], i16,
                           kind="ExternalInput")
    dstloc2 = nc.dram_tensor("dstloc2", [P, NBLK * MC2], f32,
                             kind="ExternalInput")
    invdeg2 = nc.dram_tensor("invdeg2", [P, NBLK * MC2], f32,
                             kind="ExternalInput")
    iota_in = nc.dram_tensor("iota_in", [P, P], f32, kind="ExternalInput")
    ident_in = nc.dram_tensor("ident_in", [P, P], bf16, kind="ExternalInput")
    outp = nc.dram_tensor("out", [PER_CORE, D_OUT], f32, kind="ExternalOutput")

    EXP = mybir.ActivationFunctionType.Exp
    LN = mybir.ActivationFunctionType.Ln
    RELU = mybir.ActivationFunctionType.Relu
    COPY = mybir.ActivationFunctionType.Copy
    EQ = mybir.AluOpType.is_equal
    MUL = mybir.AluOpType.mult
    SUB = mybir.AluOpType.subtract
    ADD = mybir.AluOpType.add
    MAX = mybir.AluOpType.max
    AXX = mybir.AxisListType.X

    with tile.TileContext(nc) as tc:
        with (
            tc.tile_pool(name="const", bufs=1) as cp,
            tc.tile_pool(name="gath", bufs=3) as gp,
            tc.tile_pool(name="gath2", bufs=2) as gp2,
            tc.tile_pool(name="oh1", bufs=4) as ohp,
            tc.tile_pool(name="ht", bufs=2) as htp,
            tc.tile_pool(name="meant", bufs=3) as mtp,
            tc.tile_pool(name="hout", bufs=3) as hop,
            tc.tile_pool(name="small", bufs=6) as smp,
            tc.tile_pool(name="psA", bufs=2, space="PSUM") as psA,
            tc.tile_pool(name="psO", bufs=2, space="PSUM") as psO,
            tc.tile_pool(name="psT", bufs=2, space="PSUM") as psT,
            tc.tile_pool(name="psW", bufs=2, space="PSUM") as psW,
            tc.tile_pool(name="dram", bufs=1, space="DRAM") as dram,
        ):
            # ---- constants ----
            iota_t = cp.tile([P, P], f32, tag="iota")
            nc.sync.dma_start(iota_t[:], iota_in[:])
            ident_t = cp.tile([P, P], bf16, tag="ident")
            nc.sync.dma_start(ident_t[:], ident_in[:])
            ones_t = cp.tile([1, P], bf16, tag="ones")
            nc.vector.memset(ones_t[:], 1.0)
            # warm the natural_log_exp_and_others table set so no act-table
            # reload lands mid-kernel (Ln then Exp narrows the possible set)
            warm = smp.tile([P, 1], f32, tag="warm")
            nc.scalar.activation(warm[:], iota_t[:, 1:2], LN)
            nc.scalar.activation(warm[:], warm[:], EXP)
            gidx_sb = cp.tile([P, NB * MC * 8], i16, tag="gidx")
            nc.sync.dma_start(gidx_sb[:], gidx[:])
            dstloc_sb = cp.tile([P, NB * MC], f32, tag="dstloc")
            nc.sync.dma_start(dstloc_sb[:], dstloc[:])
            invdeg_sb = cp.tile([P, NB * MC], f32, tag="invdeg")
            nc.sync.dma_start(invdeg_sb[:], invdeg[:])
            gidx2_sb = cp.tile([P, NBLK * MC2 * 8], i16, tag="gidx2")
            nc.sync.dma_start(gidx2_sb[:], gidx2[:])
            dstloc2_sb = cp.tile([P, NBLK * MC2], f32, tag="dstloc2")
            nc.sync.dma_start(dstloc2_sb[:], dstloc2[:])
            invdeg2_sb = cp.tile([P, NBLK * MC2], f32, tag="invdeg2")
            nc.sync.dma_start(invdeg2_sb[:], invdeg2[:])

            w1l_sb = cp.tile([P, D_H1], bf16, tag="w1l")
            nc.sync.dma_start(w1l_sb[:], w1l[:])
            w1r_sb = cp.tile([P, D_H1], bf16, tag="w1r")
            nc.sync.dma_start(w1r_sb[:], w1r[:])
            b1_sb = cp.tile([1, D_H1], bf16, tag="b1")
            nc.sync.dma_start(b1_sb[:], b1[:])
            b1t_sb = cp.tile([P, 2], f32, tag="b1t")
            nc.sync.dma_start(b1t_sb[:], b1t[:])

            # resident cross-phase SBUF state
            xT_res = cp.tile([P, PER_CORE], bf16, tag="xT")
            nc.sync.dma_start(xT_res[:], xownT[:])
            h1T_res = cp.tile([P, 2, PER_CORE], bf16, tag="h1T")
            h1st = cp.tile([P, NB, D_H1], bf16, tag="h1st")
            t3st = cp.tile([P, NB, P], bf16, tag="t3st")
            y_out = cp.tile([P, NB, D_OUT], f32, tag="yout")
            r3_res = cp.tile([P, NB, D_OUT], f32, tag="r3")
            y_res = cp.tile([P, NB, D_OUT], f32, tag="y")
            negm_res = cp.tile([P, NB], f32, tag="negm")
            ssum_res = cp.tile([P, NB], f32, tag="ssum")
            ls_res = cp.tile([P, NB], f32, tag="ls")
            ohc2 = cp.tile([P, NBLK * MC2, P], bf16, tag="ohc2")

            # ---- DRAM intermediates ----
            h1_own = dram.tile([PER_CORE, D_H1], bf16, tag="h1o")
            t3_own = dram.tile([PER_CORE, P], bf16, tag="t3o")
            part2 = dram.tile([NPAD, D_H1], bf16, tag="p2")
            rs1out = dram.tile([PER_CORE, D_H1], bf16, tag="rs1")
            part3 = dram.tile([NPAD, D_OUT], bf16, tag="p3")
            rs2out = dram.tile([PER_CORE, D_OUT], bf16, tag="rs2")

            def gather_block(dst_tile, src_ap, idx_sb, c0, nchunk, elem,
                             GSZ=8):
                """Gather nchunk*128 rows starting at chunk c0 of idx_sb."""
                if "nogather" in abl:
                    return
                g0 = 0
                while g0 < nchunk:
                    gsz = min(GSZ, nchunk - g0)
                    cc = (c0 + g0) * 8
                    nc.gpsimd.dma_gather(
                        dst_tile[:, g0:g0 + gsz, :], src_ap,
                        idx_sb[:, cc:cc + gsz * 8],
                        gsz * P, gsz * P, elem, single_packet=False)
                    g0 += gsz

            # ================= Layer 1 (dst-side, local) =================
            for b in range(NB if "l1" not in abl else 0):
                gath = gp.tile([P, MC, D_IN], bf16, tag="gath")
                gather_block(gath, xbf[:], gidx_sb, b * MC, MC, D_IN)
                oh1 = ohp.tile([P, MC, P], bf16, tag="oh1")
                for c in range(MC):
                    col = b * MC + c
                    nc.vector.tensor_scalar(
                        oh1[:, c, :], iota_t[:],
                        dstloc_sb[:, col:col + 1], invdeg_sb[:, col:col + 1],
                        EQ, MUL)
                agg = psA.tile([P, D_H1], f32, tag="agg")
                for c in range(MC if "noagg" not in abl else 0):
                    nc.tensor.matmul(agg[:, 0:P], gath[:, c, :], oh1[:, c, :],
                                     start=(c == 0), stop=(c == MC - 1))
                meanT = mtp.tile([P, P], bf16, tag="meanT")
                nc.scalar.activation(meanT[:], agg[:, 0:P], COPY)

                xT = xT_res[:, b * P:(b + 1) * P]

                # node-major h1 (for the L2 src-side gathers)
                op = psO.tile([P, D_H1], f32, tag="outp")
                nc.tensor.matmul(op[:], meanT[:], w1l_sb[:],
                                 start=True, stop=False)
                nc.tensor.matmul(op[:], xT, w1r_sb[:],
                                 start=False, stop=False)
                nc.tensor.matmul(op[:], ones_t[:], b1_sb[:],
                                 start=False, stop=True)
                nc.scalar.activation(h1st[:, b, :], op[:], RELU)
                # transposed h1 (for the L2 root term), SBUF-resident
                for s in range(2):
                    tph = psT.tile([P, P], f32, tag="tp")
                    nc.tensor.matmul(tph[:], w1l_sb[:, s * P:(s + 1) * P],
                                     meanT[:], start=True, stop=False)
                    nc.tensor.matmul(tph[:], w1r_sb[:, s * P:(s + 1) * P],
                                     xT, start=False, stop=True)
                    nc.scalar.activation(h1T_res[:, s, b * P:(b + 1) * P],
                                         tph[:], RELU,
                                         bias=b1t_sb[:, s:s + 1])

            nc.sync.dma_start(
                h1_own[:].rearrange("(b p) n -> p b n", p=P), h1st[:])

            # src-side one-hots for L2/L3: only need dstloc2/invdeg2, so
            # emit them here to overlap the L1 tail / L2 gather ramp on DVE
            for col in range(NBLK * MC2):
                nc.vector.tensor_scalar(
                    ohc2[:, col, :], iota_t[:],
                    dstloc2_sb[:, col:col + 1],
                    invdeg2_sb[:, col:col + 1], EQ, MUL)

            # layer-2/3 weights first needed after L1
            w2l_sb = cp.tile([P, 2, D_H2], bf16, tag="w2l")
            nc.sync.dma_start(w2l_sb[:], w2l.rearrange("(s p) n -> p s n", p=P))
            w2r_sb = cp.tile([P, 2, D_H2], bf16, tag="w2r")
            nc.sync.dma_start(w2r_sb[:], w2r.rearrange("(s p) n -> p s n", p=P))
            b2row_sb = cp.tile([1, D_H2], bf16, tag="b2row")
            nc.sync.dma_start(b2row_sb[:], b2row[:])
            w3lr_sb = cp.tile([P, 8, P], bf16, tag="w3lr")
            nc.sync.dma_start(w3lr_sb[:], w3lr.rearrange("(s p) n -> p s n", p=P))
            b3_sb = cp.tile([1, P], bf16, tag="b3")
            nc.sync.dma_start(b3_sb[:], b3pad[:])

            # ========== L2 src-side partial aggregation ==========
            # partial[dst, :] += invdeg[dst] * h1_own[srcloc, :] for local
            # edges, per dst block; one-hots built once, cached for L3.
            for gg in range((NBLK + GG - 1) // GG if "l2p" not in abl else 0):
                glo = gg * GG
                nblks = min(GG, NBLK - glo)
                gath = gp2.tile([P, GG * MC2, D_H1], bf16, tag="gath2")
                gather_block(gath, h1_own[:], gidx2_sb, glo * MC2,
                             nblks * MC2, D_H1, GSZ=GG * MC2)
                st2 = hop.tile([P, GG, D_H1], bf16, tag="st2")
                for j in range(nblks):
                    g = glo + j
                    pp = psA.tile([P, D_H1], f32, tag="agg")
                    for c in range(MC2 if "noagg" not in abl else 0):
                        nc.tensor.matmul(pp[:], ohc2[:, g * MC2 + c, :],
                                         gath[:, j * MC2 + c, :],
                                         start=(c == 0), stop=(c == MC2 - 1))
                    nc.scalar.activation(st2[:, j, :], pp[:], COPY)
                nc.sync.dma_start(
                    part2[glo * P:(glo + nblks) * P, :].rearrange(
                        "(j p) n -> p j n", p=P), st2[:])

            if "noag" not in abl:
                nc.gpsimd.collective_compute(
                    "ReduceScatter", mybir.AluOpType.add,
                    replica_groups=[list(range(NCORES))],
                    ins=[part2.opt()], outs=[rs1out.opt()])

            # ====== L2 transform fused with L3 transform ======
            nc.vector.memset(t3st[:], 0.0)
            mean2_sb = cp.tile([P, NB, D_H1], bf16, tag="mean2")
            nc.sync.dma_start(
                mean2_sb[:], rs1out[:].rearrange("(b p) n -> p b n", p=P))
            for b in range(NB if "l2t" not in abl else 0):
                # PE transpose of mean2 block -> feat-major
                meanT2 = mtp.tile([P, 2, P], bf16, tag="meanT2")
                for k in range(2):
                    pt = psT.tile([P, P], f32, tag="tp")
                    nc.tensor.matmul(pt[:],
                                     mean2_sb[:, b, k * P:(k + 1) * P],
                                     ident_t[:], start=True, stop=True)
                    nc.vector.tensor_copy(meanT2[:, k, :], pt[:])
                # h2^T slices: mean half + root half + bias, in wide PSUM so
                # one RELU covers 4 slices (bias added via ones-matmul)
                hT = htp.tile([P, 8, P], bf16, tag="hT")
                for h4 in range(2 if "noout" not in abl else 0):
                    hp4 = psW.tile([P, 4 * P], f32, tag="tp4")
                    for si in range(4):
                        s = h4 * 4 + si
                        hp = hp4[:, si * P:(si + 1) * P]
                        for k in range(2):
                            nc.tensor.matmul(
                                hp, w2l_sb[:, k, s * P:(s + 1) * P],
                                meanT2[:, k, :], start=(k == 0), stop=False)
                        for k in range(2):
                            nc.tensor.matmul(
                                hp, w2r_sb[:, k, s * P:(s + 1) * P],
                                h1T_res[:, k, b * P:(b + 1) * P],
                                start=False, stop=False)
                        nc.tensor.matmul(
                            hp, b2row_sb[0:1, s * P:(s + 1) * P], ones_t[:],
                            start=False, stop=True)
                    nc.scalar.activation(hT[:, h4 * 4:(h4 + 1) * 4, :],
                                         hp4[:], RELU)

                # [t3 | r3] = h2 @ [w3l | w3r] + [0 | b3] (lhsT = h2^T)
                trt = psO.tile([P, D_H1], f32, tag="outp")
                tr = trt[:, 0:P]
                for s in range(8):
                    nc.tensor.matmul(tr, hT[:, s, :], w3lr_sb[:, s, :],
                                     start=(s == 0), stop=False)
                nc.tensor.matmul(tr, ones_t[:], b3_sb[:],
                                 start=False, stop=True)
                nc.vector.tensor_copy(t3st[:, b, 0:D_OUT], trt[:, 0:D_OUT])
                nc.vector.tensor_copy(r3_res[:, b, :], trt[:, D_OUT:P])

            nc.sync.dma_start(
                t3_own[:].rearrange("(b p) n -> p b n", p=P), t3st[:])

            # ========== L3 src-side partial aggregation ==========
            for gg in range((NBLK + GG - 1) // GG if "l3p" not in abl else 0):
                glo = gg * GG
                nblks = min(GG, NBLK - glo)
                gath = gp2.tile([P, GG * MC2, P], bf16, tag="gath3")
                gather_block(gath, t3_own[:], gidx2_sb, glo * MC2,
                             nblks * MC2, P, GSZ=GG * MC2)
                st3 = hop.tile([P, GG, D_OUT], bf16, tag="st3")
                for j in range(nblks):
                    g = glo + j
                    pp = psA.tile([P, D_H1], f32, tag="agg")
                    for c in range(MC2 if "noagg" not in abl else 0):
                        nc.tensor.matmul(pp[:, 0:D_OUT], ohc2[:, g * MC2 + c, :],
                                         gath[:, j * MC2 + c, 0:D_OUT],
                                         start=(c == 0), stop=(c == MC2 - 1))
                    nc.scalar.activation(st3[:, j, :], pp[:, 0:D_OUT], COPY)
                nc.sync.dma_start(
                    part3[glo * P:(glo + nblks) * P, :].rearrange(
                        "(j p) n -> p j n", p=P), st3[:])

            if "noag" not in abl:
                nc.gpsimd.collective_compute(
                    "ReduceScatter", mybir.AluOpType.add,
                    replica_groups=[list(range(NCORES))],
                    ins=[part3.opt()], outs=[rs2out.opt()])

            # ================= log_softmax ==============
            mean3_sb = cp.tile([P, NB, D_OUT], bf16, tag="mean3")
            nc.sync.dma_start(
                mean3_sb[:], rs2out[:].rearrange("(b p) n -> p b n", p=P))
            for b in range(NB if "l3t" not in abl else 0):
                nc.vector.tensor_tensor(y_res[:, b, :], mean3_sb[:, b, :],
                                        r3_res[:, b, :], ADD)
                nc.vector.tensor_reduce(negm_res[:, b:b + 1], y_res[:, b, :],
                                        AXX, MAX, negate=True)
                e = smp.tile([P, D_OUT], f32, tag="e")
                nc.scalar.activation(e[:], y_res[:, b, :], EXP,
                                     bias=negm_res[:, b:b + 1],
                                     scale=1.0, accum_out=ssum_res[:, b:b + 1])
            nc.scalar.activation(ls_res[:], ssum_res[:], LN)
            for b in range(NB if "l3t" not in abl else 0):
                nc.vector.tensor_scalar(y_out[:, b, :], y_res[:, b, :],
                                        negm_res[:, b:b + 1],
                                        ls_res[:, b:b + 1], ADD, SUB)
            nc.sync.dma_start(
                outp[:].rearrange("(b p) n -> p b n", p=P), y_out[:])

    nc.compile()
    return nc


def _wrap16(a):
    """idx i -> partition i%16, col i//16; replicated to 128 partitions."""
    w = a.reshape(-1, 16).T
    return np.ascontiguousarray(np.tile(w, (8, 1)))


def _balanced_perm(deg):
    """Assign nodes to 80 blocks of 128 so block in-degree sums are even.

    Greedy: highest-degree node goes to the open block with the lowest
    degree sum. Returns newpos[old_node] -> permuted node id. This makes
    every block need exactly ceil(E/(NBLK*P)) = 16 edge chunks instead of
    the unbalanced max (17+), trimming gather + aggregation work ~6%.
    """
    import heapq
    order = np.argsort(-deg, kind="stable")
    heap = [(0, 0, g) for g in range(NBLK)]
    heapq.heapify(heap)
    newpos = np.empty(NPAD, np.int64)
    fill = np.zeros(NBLK, np.int64)
    for n in order:
        s, _, g = heapq.heappop(heap)
        newpos[n] = g * P + fill[g]
        fill[g] += 1
        if fill[g] < P:
            heapq.heappush(heap, (s + int(deg[n]), int(fill[g]), g))
    return newpos


def _prep(x, edge_index):
    src = np.asarray(edge_index[0], dtype=np.int64)
    dst = np.asarray(edge_index[1], dtype=np.int64)
    deg = np.bincount(dst, minlength=NPAD).astype(np.float64)
    invdeg_n = (1.0 / np.maximum(deg, 1.0)).astype(np.float32)

    newpos = _balanced_perm(deg)
    oldnode = np.empty(NPAD, np.int64)
    oldnode[newpos] = np.arange(NPAD)
    psrc = newpos[src]
    pdst = newpos[dst]

    # ---------- L1 dst-side organization ----------
    order = np.argsort(pdst, kind="stable")
    dsts = pdst[order]
    srcs = psrc[order]
    inv_e = invdeg_n[dst[order]]
    starts = np.searchsorted(dsts, np.arange(0, NPAD + P, P))
    cnt = starts[1:] - starts[:-1]
    MC = max(1, int(np.ceil(cnt.max() / P)))

    xp = np.zeros((NPAD, D_IN), dtype=np.float32)
    xp[:N_NODES] = x
    xp = xp[oldnode]           # permuted node order

    l1_per_core = []
    for r in range(NCORES):
        gparts, dparts, iparts = [], [], []
        for j in range(NB):
            g = r * NB + j
            lo, hi = starts[g], starts[g + 1]
            n = hi - lo
            o2 = lo + np.argsort(srcs[lo:hi], kind="stable")
            sg = np.zeros(MC * P, dtype=np.int16)
            dg = np.full(MC * P, -1.0, dtype=np.float32)
            ig = np.zeros(MC * P, dtype=np.float32)
            sg[:n] = srcs[o2].astype(np.int16)
            dg[:n] = (dsts[o2] - g * P).astype(np.float32)
            ig[:n] = inv_e[o2]
            gparts.append(_wrap16(sg))
            dparts.append(np.ascontiguousarray(dg.reshape(MC, P).T))
            iparts.append(np.ascontiguousarray(ig.reshape(MC, P).T))
        l1_per_core.append((
            np.concatenate(gparts, axis=1),
            np.concatenate(dparts, axis=1),
            np.concatenate(iparts, axis=1),
        ))

    # ---------- L2/L3 src-side organization ----------
    core_of_src = psrc // PER_CORE
    src_data = []   # per core: (srcloc, blk, dloc, iv) sorted by (blk, srcloc)
    MC2 = 1
    for r in range(NCORES):
        m = core_of_src == r
        es = psrc[m] - r * PER_CORE
        ed = pdst[m]
        iv = invdeg_n[dst[m]]
        blk = ed // P
        o2 = np.lexsort((es, blk))
        es, ed, iv, blk = es[o2], ed[o2], iv[o2], blk[o2]
        bstart = np.searchsorted(blk, np.arange(NBLK + 1))
        bcnt = bstart[1:] - bstart[:-1]
        MC2 = max(MC2, int(np.ceil(bcnt.max() / P)))
        src_data.append((es, ed, iv, blk, bstart))

    src_per_core = []
    for r in range(NCORES):
        es, ed, iv, blk, bstart = src_data[r]
        SL = MC2 * P
        sg = np.zeros(NBLK * SL, dtype=np.int16)
        dg = np.full(NBLK * SL, -1.0, dtype=np.float32)
        ig = np.zeros(NBLK * SL, dtype=np.float32)
        for g in range(NBLK):
            lo, hi = bstart[g], bstart[g + 1]
            n = hi - lo
            sg[g * SL:g * SL + n] = es[lo:hi].astype(np.int16)
            dg[g * SL:g * SL + n] = (ed[lo:hi] - g * P).astype(np.float32)
            ig[g * SL:g * SL + n] = iv[lo:hi]
        src_per_core.append((
            _wrap16(sg),
            np.ascontiguousarray(dg.reshape(NBLK * MC2, P).T),
            np.ascontiguousarray(ig.reshape(NBLK * MC2, P).T),
        ))

    return xp, l1_per_core, src_per_core, MC, MC2, newpos


def _make_in_maps(x, edge_index, w1l, w1r, b1, w2l, w2r, b2, w3l, w3r, b3):
    x = np.ascontiguousarray(np.asarray(x, dtype=np.float32))
    xp, l1_per_core, src_per_core, MC, MC2, newpos = _prep(
        x, np.asarray(edge_index))

    iota = np.tile(np.arange(P, dtype=np.float32), (P, 1))
    ident = np.eye(P, dtype=np.float32).astype(BF)
    b1v = np.asarray(b1, np.float32).reshape(-1)
    b2v = np.asarray(b2, np.float32).reshape(-1)
    xbf = xp.astype(BF)
    common = {
        "xbf": xbf,
        "w1l": np.asarray(w1l, np.float32).astype(BF),
        "w1r": np.asarray(w1r, np.float32).astype(BF),
        "b1": b1v.reshape(1, D_H1).astype(BF),
        "b1t": np.ascontiguousarray(b1v.reshape(2, P).T),
        "w2l": np.asarray(w2l, np.float32).astype(BF),
        "w2r": np.asarray(w2r, np.float32).astype(BF),
        "b2row": b2v.reshape(1, D_H2).astype(BF),
        "w3lr": np.ascontiguousarray(np.concatenate(
            [np.asarray(w3l, np.float32), np.asarray(w3r, np.float32)],
            axis=1)).astype(BF),
        "b3pad": np.concatenate(
            [np.zeros(D_OUT, np.float32),
             np.asarray(b3, np.float32).reshape(-1)]).reshape(1, P).astype(BF),
        "iota_in": iota,
        "ident_in": ident,
    }
    in_maps = []
    for r in range(NCORES):
        g1, d1, i1 = l1_per_core[r]
        g2, d2, i2 = src_per_core[r]
        m = dict(common)
        m["xownT"] = np.ascontiguousarray(
            xbf[r * PER_CORE:(r + 1) * PER_CORE].T)
        m["gidx"] = g1
        m["dstloc"] = d1
        m["invdeg"] = i1
        m["gidx2"] = g2
        m["dstloc2"] = d2
        m["invdeg2"] = i2
        in_maps.append(m)
    return in_maps, (MC, MC2), newpos


def kernel(x, edge_index, w1l, w1r, b1, w2l, w2r, b2, w3l, w3r, b3):
    global LAST_RESULTS
    import os
    from concourse.bass_utils import run_bass_kernel_spmd

    if os.environ.get("BASS_TRACE"):
        try:
            import antenv.axon_hooks  # noqa: F401
        except ImportError:
            os.environ.pop("BASS_TRACE", None)  # no NTFF hook here

    in_maps, key, newpos = _make_in_maps(x, edge_index, w1l, w1r, b1, w2l,
                                         w2r, b2, w3l, w3r, b3)
    if key not in _CACHE:
        _CACHE[key] = _build(key)
    nc = _CACHE[key]

    res = run_bass_kernel_spmd(nc, in_maps, core_ids=list(range(NCORES)))
    LAST_RESULTS = res
    out = np.concatenate([res.results[r]["out"] for r in range(NCORES)], axis=0)
    return np.ascontiguousarray(out[newpos[:N_NODES]])


# revision 12
# speedup vs baseline: 1.6089x; 1.0873x over previous
"""3-layer GraphSAGE (mean aggr) on Trainium2, 8-core SPMD, fp16 compute.

Strategy (graph/data parallel, ReduceScatter formulation):
  - Nodes padded 10000 -> 10240, degree-balanced permutation into 80
    blocks of 128 (block in-degrees ~equal so L1 needs exactly 16 edge
    chunks per block). Core r owns blocks [10r, 10r+10).
  - L1 (x replicated, no exchange): dst-side gather of x rows per owned
    block + one-hot matmul segment-mean on the PE; produces node-major
    h1_own (DRAM, for L2 gathers) and feat-major h1T (SBUF, for the L2
    root term).
  - L2/L3 use SRC-side partial aggregation + ReduceScatter instead of
    AllGather: core r takes the ~20k edges whose SOURCE it owns, grouped
    by dst block (80 groups x MC2 chunks of 128, padded), gathers its
    own h1 rows per edge, and accumulates per-dst-block partial sums
    (one-hot matmuls, invdeg folded in) into a [10240,256] fp16 partial
    tensor. One ReduceScatter(add) then gives each core the exact mean
    rows for its own 1280 nodes.  Collective cost drops ~5x vs AllGather
    (charged on the 0.66MB scattered output, not the 5.2MB gathered one).
  - L2 transform consumes the scattered mean directly (PE transpose via
    identity matmul to feat-major), folds the root half as extra
    accumulating matmuls from SBUF-resident h1T, and emits h2 transposed
    so t3|r3 = h2 @ [w3l|w3r] runs in the same block loop (h2 never in
    DRAM).  L3 repeats the src-side partial + ReduceScatter on the
    64-wide t3 (padded to 128 cols for the 256B-min gather row size).
  - log_softmax: all EXP passes first (accumulating per-block sums),
    then ONE Ln over [128,10] — avoids per-block activation-table
    thrash.  PSUM accumulation is f32 end-to-end.
"""

import numpy as np
BF = np.float16

N_NODES = 10000
NPAD = 10240
NCORES = 8
P = 128
NB = 10                      # dst blocks per core
NBLK = NPAD // P             # 80
PER_CORE = NB * P            # 1280
D_IN, D_H1, D_H2, D_OUT = 128, 256, 1024, 64

_CACHE = {}
LAST_RESULTS = None          # test harness reads exec_time_ns from here


def _build(key):
    import os
    import concourse.bacc as bacc
    import concourse.mybir as mybir
    import concourse.tile as tile

    MC, MC2, OVF = key
    abl = set(os.environ.get("KABL", "").split(","))
    GG = int(os.environ.get("KGG", "8"))      # dst blocks per src-side gather

    f32 = mybir.dt.float32
    bf16 = mybir.dt.float16
    f8 = mybir.dt.float8e4 if "nof8" not in abl else mybir.dt.float16
    i16 = mybir.dt.int16
    nc = bacc.Bacc("TRN2", target_bir_lowering=False, debug=False,
                   num_devices=NCORES)

    xbf = nc.dram_tensor("xbf", [NPAD, D_IN], bf16, kind="ExternalInput")
    xownT = nc.dram_tensor("xownT", [P, PER_CORE], bf16, kind="ExternalInput")
    w1l = nc.dram_tensor("w1l", [D_IN, D_H1], bf16, kind="ExternalInput")
    w1r = nc.dram_tensor("w1r", [D_IN, D_H1], bf16, kind="ExternalInput")
    b1 = nc.dram_tensor("b1", [1, D_H1], bf16, kind="ExternalInput")
    b1t = nc.dram_tensor("b1t", [P, 2], f32, kind="ExternalInput")
    w2l = nc.dram_tensor("w2l", [D_H1, D_H2], bf16, kind="ExternalInput")
    w2r = nc.dram_tensor("w2r", [D_H1, D_H2], bf16, kind="ExternalInput")
    b2row = nc.dram_tensor("b2row", [1, D_H2], bf16, kind="ExternalInput")
    w3lr = nc.dram_tensor("w3lr", [D_H2, P], bf16, kind="ExternalInput")
    b3pad = nc.dram_tensor("b3pad", [1, P], bf16, kind="ExternalInput")
    gidx = nc.dram_tensor("gidx", [P, NB * MC * 8], i16, kind="ExternalInput")
    dstloc = nc.dram_tensor("dstloc", [P, NB * MC], f32, kind="ExternalInput")
    invdeg = nc.dram_tensor("invdeg", [P, NB * MC], f32, kind="ExternalInput")
    gidx2 = nc.dram_tensor("gidx2", [P, NBLK * MC2 * 8], i16,
                           kind="ExternalInput")
    dstloc2 = nc.dram_tensor("dstloc2", [P, NBLK * MC2], f32,
                             kind="ExternalInput")
    invdeg2 = nc.dram_tensor("invdeg2", [P, NBLK * MC2], f32,
                             kind="ExternalInput")
    gidx3 = nc.dram_tensor("gidx3", [P, OVF * 8], i16, kind="ExternalInput")
    sidx3 = nc.dram_tensor("sidx3", [P, OVF * 8], i16, kind="ExternalInput")
    ivov = nc.dram_tensor("ivov", [P, OVF], f32, kind="ExternalInput")
    iota_in = nc.dram_tensor("iota_in", [P, P], f32, kind="ExternalInput")
    ident_in = nc.dram_tensor("ident_in", [P, P], bf16, kind="ExternalInput")
    outp = nc.dram_tensor("out", [PER_CORE, D_OUT], f32, kind="ExternalOutput")

    EXP = mybir.ActivationFunctionType.Exp
    LN = mybir.ActivationFunctionType.Ln
    RELU = mybir.ActivationFunctionType.Relu
    COPY = mybir.ActivationFunctionType.Copy
    EQ = mybir.AluOpType.is_equal
    MUL = mybir.AluOpType.mult
    SUB = mybir.AluOpType.subtract
    ADD = mybir.AluOpType.add
    MAX = mybir.AluOpType.max
    AXX = mybir.AxisListType.X

    with tile.TileContext(nc) as tc:
        with (
            tc.tile_pool(name="const", bufs=1) as cp,
            tc.tile_pool(name="gath", bufs=3) as gp,
            tc.tile_pool(name="gath2", bufs=2) as gp2,
            tc.tile_pool(name="oh1", bufs=4) as ohp,
            tc.tile_pool(name="ht", bufs=2) as htp,
            tc.tile_pool(name="meant", bufs=3) as mtp,
            tc.tile_pool(name="hout", bufs=3) as hop,
            tc.tile_pool(name="small", bufs=6) as smp,
            tc.tile_pool(name="psA", bufs=2, space="PSUM") as psA,
            tc.tile_pool(name="psO", bufs=2, space="PSUM") as psO,
            tc.tile_pool(name="psT", bufs=2, space="PSUM") as psT,
            tc.tile_pool(name="psW", bufs=2, space="PSUM") as psW,
            tc.tile_pool(name="dram", bufs=1, space="DRAM") as dram,
        ):
            # ---- constants ----
            iota_t = cp.tile([P, P], f32, tag="iota")
            nc.sync.dma_start(iota_t[:], iota_in[:])
            ident_t = cp.tile([P, P], bf16, tag="ident")
            nc.sync.dma_start(ident_t[:], ident_in[:])
            ones_t = cp.tile([1, P], bf16, tag="ones")
            nc.vector.memset(ones_t[:], 1.0)
            # warm the natural_log_exp_and_others table set so no act-table
            # reload lands mid-kernel (Ln then Exp narrows the possible set)
            warm = smp.tile([P, 1], f32, tag="warm")
            nc.scalar.activation(warm[:], iota_t[:, 1:2], LN)
            nc.scalar.activation(warm[:], warm[:], EXP)
            gidx_sb = cp.tile([P, NB * MC * 8], i16, tag="gidx")
            nc.sync.dma_start(gidx_sb[:], gidx[:])
            dstloc_sb = cp.tile([P, NB * MC], f32, tag="dstloc")
            nc.sync.dma_start(dstloc_sb[:], dstloc[:])
            invdeg_sb = cp.tile([P, NB * MC], f32, tag="invdeg")
            nc.sync.dma_start(invdeg_sb[:], invdeg[:])
            gidx2_sb = cp.tile([P, NBLK * MC2 * 8], i16, tag="gidx2")
            nc.sync.dma_start(gidx2_sb[:], gidx2[:])
            gidx3_sb = cp.tile([P, OVF * 8], i16, tag="gidx3")
            nc.sync.dma_start(gidx3_sb[:], gidx3[:])
            sidx3_sb = cp.tile([P, OVF * 8], i16, tag="sidx3")
            nc.sync.dma_start(sidx3_sb[:], sidx3[:])
            ivov_sb = cp.tile([P, OVF], f32, tag="ivov")
            nc.sync.dma_start(ivov_sb[:], ivov[:])
            dstloc2_sb = cp.tile([P, NBLK * MC2], f32, tag="dstloc2")
            nc.sync.dma_start(dstloc2_sb[:], dstloc2[:])
            invdeg2_sb = cp.tile([P, NBLK * MC2], f32, tag="invdeg2")
            nc.sync.dma_start(invdeg2_sb[:], invdeg2[:])

            w1l_sb = cp.tile([P, D_H1], bf16, tag="w1l")
            nc.sync.dma_start(w1l_sb[:], w1l[:])
            w1r_sb = cp.tile([P, D_H1], bf16, tag="w1r")
            nc.sync.dma_start(w1r_sb[:], w1r[:])
            b1_sb = cp.tile([1, D_H1], bf16, tag="b1")
            nc.sync.dma_start(b1_sb[:], b1[:])
            b1t_sb = cp.tile([P, 2], f32, tag="b1t")
            nc.sync.dma_start(b1t_sb[:], b1t[:])

            # resident cross-phase SBUF state
            xT_res = cp.tile([P, PER_CORE], bf16, tag="xT")
            nc.sync.dma_start(xT_res[:], xownT[:])
            h1T_res = cp.tile([P, 2, PER_CORE], bf16, tag="h1T")
            h1st = cp.tile([P, NB, D_H1], f8, tag="h1st")
            t3st = cp.tile([P, NB, P], bf16, tag="t3st")
            y_out = cp.tile([P, NB, D_OUT], f32, tag="yout")
            r3_res = cp.tile([P, NB, D_OUT], f32, tag="r3")
            y_res = cp.tile([P, NB, D_OUT], f32, tag="y")
            negm_res = cp.tile([P, NB], f32, tag="negm")
            ssum_res = cp.tile([P, NB], f32, tag="ssum")
            ls_res = cp.tile([P, NB], f32, tag="ls")
            ohc2 = cp.tile([P, NBLK * MC2, P], bf16, tag="ohc2")

            # ---- DRAM intermediates ----
            h1_own = dram.tile([PER_CORE, D_H1], f8, tag="h1o")
            t3_own = dram.tile([PER_CORE, P], bf16, tag="t3o")
            part2 = dram.tile([NPAD, D_H1], bf16, tag="p2")
            rs1out = dram.tile([PER_CORE, D_H1], bf16, tag="rs1")
            part3 = dram.tile([NPAD, P], bf16, tag="p3")
            rs2out = dram.tile([PER_CORE, P], bf16, tag="rs2")

            def gather_block(dst_tile, src_ap, idx_sb, c0, nchunk, elem,
                             GSZ=8):
                """Gather nchunk*128 rows starting at chunk c0 of idx_sb."""
                if "nogather" in abl:
                    return
                g0 = 0
                while g0 < nchunk:
                    gsz = min(GSZ, nchunk - g0)
                    cc = (c0 + g0) * 8
                    nc.gpsimd.dma_gather(
                        dst_tile[:, g0:g0 + gsz, :], src_ap,
                        idx_sb[:, cc:cc + gsz * 8],
                        gsz * P, gsz * P, elem, single_packet=False)
                    g0 += gsz

            # ================= Layer 1 (dst-side, local) =================
            for b in range(NB if "l1" not in abl else 0):
                gath = gp.tile([P, MC, D_IN], bf16, tag="gath")
                gather_block(gath, xbf[:], gidx_sb, b * MC, MC, D_IN)
                oh1 = ohp.tile([P, MC, P], bf16, tag="oh1")
                for c in range(MC):
                    col = b * MC + c
                    nc.vector.tensor_scalar(
                        oh1[:, c, :], iota_t[:],
                        dstloc_sb[:, col:col + 1], invdeg_sb[:, col:col + 1],
                        EQ, MUL)
                agg = psA.tile([P, D_H1], f32, tag="agg")
                for c in range(MC if "noagg" not in abl else 0):
                    nc.tensor.matmul(agg[:, 0:P], gath[:, c, :], oh1[:, c, :],
                                     start=(c == 0), stop=(c == MC - 1))
                meanT = mtp.tile([P, P], bf16, tag="meanT")
                nc.scalar.activation(meanT[:], agg[:, 0:P], COPY)

                xT = xT_res[:, b * P:(b + 1) * P]

                # node-major h1 (for the L2 src-side gathers)
                op = psO.tile([P, D_H1], f32, tag="outp")
                nc.tensor.matmul(op[:], meanT[:], w1l_sb[:],
                                 start=True, stop=False)
                nc.tensor.matmul(op[:], xT, w1r_sb[:],
                                 start=False, stop=False)
                nc.tensor.matmul(op[:], ones_t[:], b1_sb[:],
                                 start=False, stop=True)
                nc.scalar.activation(h1st[:, b, :], op[:], RELU)
                # transposed h1 (for the L2 root term), SBUF-resident
                for s in range(2):
                    tph = psT.tile([P, P], f32, tag="tp")
                    nc.tensor.matmul(tph[:], w1l_sb[:, s * P:(s + 1) * P],
                                     meanT[:], start=True, stop=False)
                    nc.tensor.matmul(tph[:], w1r_sb[:, s * P:(s + 1) * P],
                                     xT, start=False, stop=True)
                    nc.scalar.activation(h1T_res[:, s, b * P:(b + 1) * P],
                                         tph[:], RELU,
                                         bias=b1t_sb[:, s:s + 1])

            nc.sync.dma_start(
                h1_own[:].rearrange("(b p) n -> p b n", p=P), h1st[:])

            # src-side one-hots for L2/L3: only need dstloc2/invdeg2, so
            # emit them here to overlap the L1 tail / L2 gather ramp on DVE
            for col in range(NBLK * MC2):
                nc.vector.tensor_scalar(
                    ohc2[:, col, :], iota_t[:],
                    dstloc2_sb[:, col:col + 1],
                    invdeg2_sb[:, col:col + 1], EQ, MUL)

            # layer-2/3 weights first needed after L1
            w2l_sb = cp.tile([P, 2, D_H2], bf16, tag="w2l")
            nc.sync.dma_start(w2l_sb[:], w2l.rearrange("(s p) n -> p s n", p=P))
            w2r_sb = cp.tile([P, 2, D_H2], bf16, tag="w2r")
            nc.sync.dma_start(w2r_sb[:], w2r.rearrange("(s p) n -> p s n", p=P))
            b2row_sb = cp.tile([1, D_H2], bf16, tag="b2row")
            nc.sync.dma_start(b2row_sb[:], b2row[:])
            w3lr_sb = cp.tile([P, 8, P], bf16, tag="w3lr")
            nc.sync.dma_start(w3lr_sb[:], w3lr.rearrange("(s p) n -> p s n", p=P))
            b3_sb = cp.tile([1, P], bf16, tag="b3")
            nc.sync.dma_start(b3_sb[:], b3pad[:])

            # ========== L2 src-side partial aggregation ==========
            # partial[dst, :] += invdeg[dst] * h1_own[srcloc, :] for local
            # edges, per dst block; one-hots built once, cached for L3.
            for gg in range((NBLK + GG - 1) // GG if "l2p" not in abl else 0):
                glo = gg * GG
                nblks = min(GG, NBLK - glo)
                gath = gp2.tile([P, GG * MC2, D_H1], f8, tag="gath2")
                gather_block(gath, h1_own[:], gidx2_sb, glo * MC2,
                             nblks * MC2, D_H1, GSZ=GG * MC2)
                st2 = hop.tile([P, GG, D_H1], bf16, tag="st2")
                for j in range(nblks):
                    g = glo + j
                    pp = psA.tile([P, D_H1], f32, tag="agg")
                    for c in range(MC2 if "noagg" not in abl else 0):
                        nc.tensor.matmul(pp[:], ohc2[:, g * MC2 + c, :],
                                         gath[:, j * MC2 + c, :],
                                         start=(c == 0), stop=(c == MC2 - 1))
                    nc.scalar.activation(st2[:, j, :], pp[:], COPY)
                nc.sync.dma_start(
                    part2[glo * P:(glo + nblks) * P, :].rearrange(
                        "(j p) n -> p j n", p=P), st2[:])

            if "noovf" not in abl and "noovf2" not in abl:
                govf = gp2.tile([P, OVF, D_H1], f8, tag="govf")
                gather_block(govf, h1_own[:], gidx3_sb, 0, OVF, D_H1,
                             GSZ=OVF)
                sc2 = hop.tile([P, OVF, D_H1], bf16, tag="sc2")
                for c in range(OVF):
                    nc.vector.tensor_scalar(sc2[:, c, :], govf[:, c, :],
                                            ivov_sb[:, c:c + 1], None, MUL)
                nc.gpsimd.dma_scatter_add(
                    part2[:], sc2[:], sidx3_sb[:], OVF * P, OVF * P, D_H1,
                    single_packet=True)

            if "noag" not in abl:
                nc.gpsimd.collective_compute(
                    "ReduceScatter", mybir.AluOpType.add,
                    replica_groups=[list(range(NCORES))],
                    ins=[part2.opt()], outs=[rs1out.opt()])

            # ====== L2 transform fused with L3 transform ======
            nc.vector.memset(t3st[:], 0.0)
            mean2_sb = cp.tile([P, NB, D_H1], bf16, tag="mean2")
            nc.sync.dma_start(
                mean2_sb[:], rs1out[:].rearrange("(b p) n -> p b n", p=P))
            for b in range(NB if "l2t" not in abl else 0):
                # PE transpose of mean2 block -> feat-major
                meanT2 = mtp.tile([P, 2, P], bf16, tag="meanT2")
                for k in range(2):
                    pt = psT.tile([P, P], f32, tag="tp")
                    nc.tensor.matmul(pt[:],
                                     mean2_sb[:, b, k * P:(k + 1) * P],
                                     ident_t[:], start=True, stop=True)
                    nc.vector.tensor_copy(meanT2[:, k, :], pt[:])
                # h2^T slices: mean half + root half + bias, in wide PSUM so
                # one RELU covers 4 slices (bias added via ones-matmul)
                hT = htp.tile([P, 8, P], bf16, tag="hT")
                for h4 in range(2 if "noout" not in abl else 0):
                    hp4 = psW.tile([P, 4 * P], f32, tag="tp4")
                    for si in range(4):
                        s = h4 * 4 + si
                        hp = hp4[:, si * P:(si + 1) * P]
                        for k in range(2):
                            nc.tensor.matmul(
                                hp, w2l_sb[:, k, s * P:(s + 1) * P],
                                meanT2[:, k, :], start=(k == 0), stop=False)
                        for k in range(2):
                            nc.tensor.matmul(
                                hp, w2r_sb[:, k, s * P:(s + 1) * P],
                                h1T_res[:, k, b * P:(b + 1) * P],
                                start=False, stop=False)
                        nc.tensor.matmul(
                            hp, b2row_sb[0:1, s * P:(s + 1) * P], ones_t[:],
                            start=False, stop=True)
                    nc.scalar.activation(hT[:, h4 * 4:(h4 + 1) * 4, :],
                                         hp4[:], RELU)

                # [t3 | r3] = h2 @ [w3l | w3r] + [0 | b3] (lhsT = h2^T)
                trt = psO.tile([P, D_H1], f32, tag="outp")
                tr = trt[:, 0:P]
                for s in range(8):
                    nc.tensor.matmul(tr, hT[:, s, :], w3lr_sb[:, s, :],
                                     start=(s == 0), stop=False)
                nc.tensor.matmul(tr, ones_t[:], b3_sb[:],
                                 start=False, stop=True)
                nc.vector.tensor_copy(t3st[:, b, 0:D_OUT], trt[:, 0:D_OUT])
                nc.vector.tensor_copy(r3_res[:, b, :], trt[:, D_OUT:P])

            nc.sync.dma_start(
                t3_own[:].rearrange("(b p) n -> p b n", p=P), t3st[:])

            # ========== L3 src-side partial aggregation ==========
            for gg in range((NBLK + GG - 1) // GG if "l3p" not in abl else 0):
                glo = gg * GG
                nblks = min(GG, NBLK - glo)
                gath = gp2.tile([P, GG * MC2, P], bf16, tag="gath3")
                gather_block(gath, t3_own[:], gidx2_sb, glo * MC2,
                             nblks * MC2, P, GSZ=GG * MC2)
                st3 = hop.tile([P, GG, D_OUT], bf16, tag="st3")
                for j in range(nblks):
                    g = glo + j
                    pp = psA.tile([P, D_H1], f32, tag="agg")
                    for c in range(MC2 if "noagg" not in abl else 0):
                        nc.tensor.matmul(pp[:, 0:D_OUT], ohc2[:, g * MC2 + c, :],
                                         gath[:, j * MC2 + c, 0:D_OUT],
                                         start=(c == 0), stop=(c == MC2 - 1))
                    nc.scalar.activation(st3[:, j, :], pp[:, 0:D_OUT], COPY)
                nc.sync.dma_start(
                    part3[glo * P:(glo + nblks) * P, 0:D_OUT].rearrange(
                        "(j p) n -> p j n", p=P), st3[:])

            if "noovf" not in abl and "noovf3" not in abl:
                govf3 = gp2.tile([P, OVF, P], bf16, tag="govf3")
                gather_block(govf3, t3_own[:], gidx3_sb, 0, OVF, P, GSZ=OVF)
                sc3 = hop.tile([P, OVF, D_OUT], bf16, tag="sc3")
                for c in range(OVF):
                    nc.vector.tensor_scalar(sc3[:, c, :],
                                            govf3[:, c, 0:D_OUT],
                                            ivov_sb[:, c:c + 1], None, MUL)
                nc.gpsimd.dma_scatter_add(
                    part3[:, 0:D_OUT], sc3[:], sidx3_sb[:], OVF * P, OVF * P,
                    D_OUT, elem_step=P, single_packet=True)

            if "noag" not in abl:
                nc.gpsimd.collective_compute(
                    "ReduceScatter", mybir.AluOpType.add,
                    replica_groups=[list(range(NCORES))],
                    ins=[part3.opt()], outs=[rs2out.opt()])

            # ================= log_softmax ==============
            mean3_sb = cp.tile([P, NB, D_OUT], bf16, tag="mean3")
            nc.sync.dma_start(
                mean3_sb[:],
                rs2out[:, 0:D_OUT].rearrange("(b p) n -> p b n", p=P))
            for b in range(NB if "l3t" not in abl else 0):
                nc.vector.tensor_tensor(y_res[:, b, :], mean3_sb[:, b, :],
                                        r3_res[:, b, :], ADD)
                nc.vector.tensor_reduce(negm_res[:, b:b + 1], y_res[:, b, :],
                                        AXX, MAX, negate=True)
                e = smp.tile([P, D_OUT], f32, tag="e")
                nc.scalar.activation(e[:], y_res[:, b, :], EXP,
                                     bias=negm_res[:, b:b + 1],
                                     scale=1.0, accum_out=ssum_res[:, b:b + 1])
            nc.scalar.activation(ls_res[:], ssum_res[:], LN)
            for b in range(NB if "l3t" not in abl else 0):
                nc.vector.tensor_scalar(y_out[:, b, :], y_res[:, b, :],
                                        negm_res[:, b:b + 1],
                                        ls_res[:, b:b + 1], ADD, SUB)
            nc.sync.dma_start(
                outp[:].rearrange("(b p) n -> p b n", p=P), y_out[:])

    nc.compile()
    return nc


def _wrap16(a):
    """idx i -> partition i%16, col i//16; replicated to 128 partitions."""
    w = a.reshape(-1, 16).T
    return np.ascontiguousarray(np.tile(w, (8, 1)))


def _balanced_perm(deg):
    """Assign nodes to 80 blocks of 128 so block in-degree sums are even.

    Greedy: highest-degree node goes to the open block with the lowest
    degree sum. Returns newpos[old_node] -> permuted node id. This makes
    every block need exactly ceil(E/(NBLK*P)) = 16 edge chunks instead of
    the unbalanced max (17+), trimming gather + aggregation work ~6%.
    """
    import heapq
    order = np.argsort(-deg, kind="stable")
    heap = [(0, 0, g) for g in range(NBLK)]
    heapq.heapify(heap)
    newpos = np.empty(NPAD, np.int64)
    fill = np.zeros(NBLK, np.int64)
    for n in order:
        s, _, g = heapq.heappop(heap)
        newpos[n] = g * P + fill[g]
        fill[g] += 1
        if fill[g] < P:
            heapq.heappush(heap, (s + int(deg[n]), int(fill[g]), g))
    return newpos


def _prep(x, edge_index):
    src = np.asarray(edge_index[0], dtype=np.int64)
    dst = np.asarray(edge_index[1], dtype=np.int64)
    deg = np.bincount(dst, minlength=NPAD).astype(np.float64)
    invdeg_n = (1.0 / np.maximum(deg, 1.0)).astype(np.float32)

    newpos = _balanced_perm(deg)
    oldnode = np.empty(NPAD, np.int64)
    oldnode[newpos] = np.arange(NPAD)
    psrc = newpos[src]
    pdst = newpos[dst]

    # ---------- L1 dst-side organization ----------
    order = np.argsort(pdst, kind="stable")
    dsts = pdst[order]
    srcs = psrc[order]
    inv_e = invdeg_n[dst[order]]
    starts = np.searchsorted(dsts, np.arange(0, NPAD + P, P))
    cnt = starts[1:] - starts[:-1]
    MC = max(1, int(np.ceil(cnt.max() / P)))

    xp = np.zeros((NPAD, D_IN), dtype=np.float32)
    xp[:N_NODES] = x
    xp = xp[oldnode]           # permuted node order

    l1_per_core = []
    for r in range(NCORES):
        gparts, dparts, iparts = [], [], []
        for j in range(NB):
            g = r * NB + j
            lo, hi = starts[g], starts[g + 1]
            n = hi - lo
            o2 = lo + np.argsort(srcs[lo:hi], kind="stable")
            sg = np.zeros(MC * P, dtype=np.int16)
            dg = np.full(MC * P, -1.0, dtype=np.float32)
            ig = np.zeros(MC * P, dtype=np.float32)
            sg[:n] = srcs[o2].astype(np.int16)
            dg[:n] = (dsts[o2] - g * P).astype(np.float32)
            ig[:n] = inv_e[o2]
            gparts.append(_wrap16(sg))
            dparts.append(np.ascontiguousarray(dg.reshape(MC, P).T))
            iparts.append(np.ascontiguousarray(ig.reshape(MC, P).T))
        l1_per_core.append((
            np.concatenate(gparts, axis=1),
            np.concatenate(dparts, axis=1),
            np.concatenate(iparts, axis=1),
        ))

    # ---------- L2/L3 src-side organization ----------
    # Chunks per (core, dst-block) cell capped at MC2=2; edges beyond
    # 2*128 per cell go to a per-core overflow list handled by
    # gather+scale+dma_scatter_add (data-dependent dst).
    core_of_src = psrc // PER_CORE
    MC2 = 2
    CAP = MC2 * P
    src_data = []
    OVF = 1
    for r in range(NCORES):
        m = core_of_src == r
        es = psrc[m] - r * PER_CORE
        ed = pdst[m]
        iv = invdeg_n[dst[m]]
        blk = ed // P
        o2 = np.lexsort((es, blk))
        es, ed, iv, blk = es[o2], ed[o2], iv[o2], blk[o2]
        bstart = np.searchsorted(blk, np.arange(NBLK + 1))
        bcnt = bstart[1:] - bstart[:-1]
        novf = int(np.maximum(bcnt - CAP, 0).sum())
        OVF = max(OVF, int(np.ceil(novf / P)))
        src_data.append((es, ed, iv, bstart))

    src_per_core = []
    for r in range(NCORES):
        es, ed, iv, bstart = src_data[r]
        SL = CAP
        sg = np.zeros(NBLK * SL, dtype=np.int16)
        dg = np.full(NBLK * SL, -1.0, dtype=np.float32)
        ig = np.zeros(NBLK * SL, dtype=np.float32)
        oes, oed, oiv = [], [], []
        for g in range(NBLK):
            lo, hi = bstart[g], bstart[g + 1]
            cnt = hi - lo
            if cnt <= CAP:
                sg[g * SL:g * SL + cnt] = es[lo:hi].astype(np.int16)
                dg[g * SL:g * SL + cnt] = (ed[lo:hi] - g * P).astype(np.float32)
                ig[g * SL:g * SL + cnt] = iv[lo:hi]
                continue
            # Pick cnt-CAP overflow edges with DISTINCT dst nodes (one per
            # dst): dma_scatter_add does racy read-modify-write per row, so
            # a scatter batch must never hit the same row twice.
            need = cnt - CAP
            dloc = ed[lo:hi] - g * P
            seen = set()
            ovf_sel = []
            for i in range(cnt):
                d = int(dloc[i])
                if d not in seen:
                    seen.add(d)
                    ovf_sel.append(i)
                    if len(ovf_sel) == need:
                        break
            assert len(ovf_sel) == need, (r, g, need, len(ovf_sel))
            sel = np.zeros(cnt, bool)
            sel[ovf_sel] = True
            oes.append(es[lo:hi][sel])
            oed.append(ed[lo:hi][sel])
            oiv.append(iv[lo:hi][sel])
            keep = ~sel
            sg[g * SL:g * SL + CAP] = es[lo:hi][keep].astype(np.int16)
            dg[g * SL:g * SL + CAP] = dloc[keep].astype(np.float32)
            ig[g * SL:g * SL + CAP] = iv[lo:hi][keep]
        oes = np.concatenate(oes) if oes else np.zeros(0, np.int64)
        oed = np.concatenate(oed) if oed else np.zeros(0, np.int64)
        oiv = np.concatenate(oiv) if oiv else np.zeros(0, np.float32)
        o3 = np.argsort(oes, kind="stable")
        oes, oed, oiv = oes[o3], oed[o3], oiv[o3]
        og = np.zeros(OVF * P, dtype=np.int16)
        osd = np.zeros(OVF * P, dtype=np.int16)
        oig = np.zeros(OVF * P, dtype=np.float32)
        og[:len(oes)] = oes.astype(np.int16)
        osd[:len(oed)] = oed.astype(np.int16)
        oig[:len(oiv)] = oiv
        src_per_core.append((
            _wrap16(sg),
            np.ascontiguousarray(dg.reshape(NBLK * MC2, P).T),
            np.ascontiguousarray(ig.reshape(NBLK * MC2, P).T),
            _wrap16(og),
            _wrap16(osd),
            np.ascontiguousarray(oig.reshape(OVF, P).T),
        ))

    return xp, l1_per_core, src_per_core, MC, MC2, OVF, newpos


def _make_in_maps(x, edge_index, w1l, w1r, b1, w2l, w2r, b2, w3l, w3r, b3):
    x = np.ascontiguousarray(np.asarray(x, dtype=np.float32))
    xp, l1_per_core, src_per_core, MC, MC2, OVF, newpos = _prep(
        x, np.asarray(edge_index))

    iota = np.tile(np.arange(P, dtype=np.float32), (P, 1))
    ident = np.eye(P, dtype=np.float32).astype(BF)
    b1v = np.asarray(b1, np.float32).reshape(-1)
    b2v = np.asarray(b2, np.float32).reshape(-1)
    xbf = xp.astype(BF)
    common = {
        "xbf": xbf,
        "w1l": np.asarray(w1l, np.float32).astype(BF),
        "w1r": np.asarray(w1r, np.float32).astype(BF),
        "b1": b1v.reshape(1, D_H1).astype(BF),
        "b1t": np.ascontiguousarray(b1v.reshape(2, P).T),
        "w2l": np.asarray(w2l, np.float32).astype(BF),
        "w2r": np.asarray(w2r, np.float32).astype(BF),
        "b2row": b2v.reshape(1, D_H2).astype(BF),
        "w3lr": np.ascontiguousarray(np.concatenate(
            [np.asarray(w3l, np.float32), np.asarray(w3r, np.float32)],
            axis=1)).astype(BF),
        "b3pad": np.concatenate(
            [np.zeros(D_OUT, np.float32),
             np.asarray(b3, np.float32).reshape(-1)]).reshape(1, P).astype(BF),
        "iota_in": iota,
        "ident_in": ident,
    }
    in_maps = []
    for r in range(NCORES):
        g1, d1, i1 = l1_per_core[r]
        g2, d2, i2, g3, s3, i3 = src_per_core[r]
        m = dict(common)
        m["xownT"] = np.ascontiguousarray(
            xbf[r * PER_CORE:(r + 1) * PER_CORE].T)
        m["gidx"] = g1
        m["dstloc"] = d1
        m["invdeg"] = i1
        m["gidx2"] = g2
        m["dstloc2"] = d2
        m["invdeg2"] = i2
        m["gidx3"] = g3
        m["sidx3"] = s3
        m["ivov"] = i3
        in_maps.append(m)
    return in_maps, (MC, MC2, OVF), newpos


def kernel(x, edge_index, w1l, w1r, b1, w2l, w2r, b2, w3l, w3r, b3):
    global LAST_RESULTS
    import os
    from concourse.bass_utils import run_bass_kernel_spmd

    if os.environ.get("BASS_TRACE"):
        try:
            import antenv.axon_hooks  # noqa: F401
        except ImportError:
            os.environ.pop("BASS_TRACE", None)  # no NTFF hook here

    in_maps, key, newpos = _make_in_maps(x, edge_index, w1l, w1r, b1, w2l,
                                         w2r, b2, w3l, w3r, b3)
    if key not in _CACHE:
        _CACHE[key] = _build(key)
    nc = _CACHE[key]

    res = run_bass_kernel_spmd(nc, in_maps, core_ids=list(range(NCORES)))
    LAST_RESULTS = res
    out = np.concatenate([res.results[r]["out"] for r in range(NCORES)], axis=0)
    return np.ascontiguousarray(out[newpos[:N_NODES]])
